# revision 9
# baseline (speedup 1.0000x reference)
"""HEPT sparse attention for Trainium2 — 8-core SPMD Bass kernel.

Reference computation (per hash-round r, head h):
  hash q/k via shared projection, argsort, gather into blocks of 128,
  blocked RBF attention: so = exp(-0.5*||q_i-k_j||^2) @ v.

Strategy (MODE="allact", the measured-fastest variant):
  - Host: bitwise-exact hash + argsort (jax CPU, identical ops to the
    reference), gather, fp16 quantization (data rows pre-scaled by
    sqrt(log2e) so the device Gram is in log2 units), per-group layout
    packing (q|k|v chunks of 1408 cols per 16-block group).
  - Device (per core, 3 of the 24 (r,h) units = 768 blocks): the whole
    log2e*-0.5*||q-k||^2 Gram of one block is ONE K=32 fp16 matmul
    (rows 0-26 data, rows 27-30 squared-norm terms hi/lo paired with
    ones rows); 4 blocks share the PE via row tiling into 4 PSUM banks.
    exp runs as one 2048-col ScalarE ACT instruction per group
    (exp(ln2*P) = 2^P); mm2 (so = A @ v, A fp16 weights) writes back
    into exp-consumed PSUM; VectorE casts f32->f16 for the output DMA.
  - Pipeline: per-group input DMAs prefetched 6 groups ahead, the exp
    ACT table preloaded during the DMA prefill, and mm2 + cast + output
    DMA emitted one group behind the fills, so every wait is pre-posted
    and ScalarE (the bottleneck engine, ~94 us busy) runs back-to-back.
  - A "split2" variant (custom-DVE 2-pass exp for a quarter of the
    Gram on VectorE) measured consistently slower (~127-132 us vs
    ~123 us) due to cross-engine handoff latency on the extra PSUM
    recycle chains, and is kept for reference.
"""

import math
import os
from contextlib import ExitStack

import numpy as np

# ---- problem constants (hardcoded; kernel.py must be self-contained) ----
N_HASHES = 3
N_HEADS = 8
PADDED_SIZE = 32768
BLOCK = 128
DIM_PER_HEAD = 24
D_QK = 27
NB = PADDED_SIZE // BLOCK          # 256 blocks per (r,h)
N_CORES = 8
UNITS = N_HASHES * N_HEADS         # 24 independent (r,h) units
UPC = UNITS // N_CORES             # 3 units per core
NBLK = UPC * NB                    # 768 blocks per core
NQUAD = NBLK // 4                  # 192 quads per core (4 blocks/quad)
SUPER_Q = 16                       # quads per super-tile (64 blocks)
KROWS = 32                         # stacked contraction rows per block

LOG2E = 1.4426950408889634
SQRT_LOG2E = math.sqrt(LOG2E)
LN2 = 0.6931471805599453

# custom-DVE exp constants
DVE_M = 12582912.0                 # 1.5 * 2^23 round-to-int magic
DVE_CLO = DVE_M - 126.0            # clamp: K >= -126
DVE_E23 = 8388608.0                # 2^23
DVE_A = 0.24973141119916378        # minimax A*((s+c)^2 + c^2) ~ 2^s, |s|<=.5
DVE_C = 1.414837949227267

NACT_BLK = 12                      # blocks/group exp'd on ACT (lanes 0-2)
NDVE_BLK = 4                       # blocks/group exp'd on DVE (lane 3)
CAST_ACT = 0                       # output cols cast on ACT (0 = all on DVE)
MODE = os.environ.get("HEPT_MODE", "split1p")

# ---- split1p mode: one-pass DVE bits-exp + ACT exp column split ----
# Gram G = 128*log2(e)*(-0.5*||q-k||^2) + 16320 (offset baked via norm rows).
# ACT path: exp(G*ln2/128 - 127.5*ln2) = 2^P2, written bf16.
# DVE path: one 8-slice custom op emits the uint16 BIT PATTERN of the bf16
#   value 2^P2 (magic-add exponent extraction + minimax quadratic mantissa),
#   negative (underflow) results clamp to 0 via the saturating uint16 store.
SC2 = math.sqrt(128.0 * LOG2E)     # host pre-scale for q/k data rows
G_OFS = 16320.0                    # = 128*127.5, split 8160 per side
BETA_1P = 0.00268750865            # minimax quad: beta*(y+H)^2 + kappa
H_1P = 121.124593
KAPPA_1P = -39.1119528
C0_MAGIC = 1.5 * 2**30
C1_VERT = H_1P + 64.0
SRC1_K = KAPPA_1P - 128.0
ACT_COLS = int(os.environ.get("HEPT_ACT_COLS", "1536"))  # ACT exp col span
DVE_COLS = 2048 - ACT_COLS         # DVE one-pass exp col span (lane-3 side)

_NC_CACHE = {}


# columns per super-tile in the combined input tensor (fp16):
#   q-stacks SUPER_Q*128 | k-stacks SUPER_Q*128 | v SUPER_Q*4*24
SUP_Q_COLS = SUPER_Q * 128
SUP_V_COLS = SUPER_Q * 4 * 24
SUP_COLS = 2 * SUP_Q_COLS + SUP_V_COLS

# per-GROUP input layout (16 blocks): q 4*128 | k 4*128 | v 16*24
GRP_Q = 512
GRP_V = 384
GRP_COLS = 2 * GRP_Q + GRP_V       # 1408
NGROUPS = NQUAD // 4               # 48 groups per core


def _register_exp_ops():
    """Register the two custom-DVE exp ops into concourse.dve_ops.

    pass1 (EXP2K_HEPT): out_i32 = (maxx(P + M, M-126) - (M-127)) * 2^23
      = (clamp(round(P), -126..) + 127) * 2^23 — exactly the fp32 bit
      pattern of 2^K, materialised by the f32->int32 store convert.
    pass2 (EXP2P_HEPT): out_f16 = A*((frac + c)^2 + c^2) * Src1 where
      frac = P - round(P) recomputed via the same magic, Src1 = the f32
      view of pass1's output. Minimax max rel err 2.55e-3.
    """
    from concourse import dve_ops
    from concourse.dve_spec import (
        Spec, Src0, Src1, C0, C1, C2, One, lower, maxx, sq,
        _has_src1,
    )
    from concourse.dve_uop import DveOpSpec

    if "EXP2K_HEPT" in dve_ops._SUB_OPCODE_FOR_NAME:
        by_name = {op.name: op for op in dve_ops.OPS}
        return by_name["EXP2K_HEPT"], by_name["EXP2P_HEPT"]

    def mk(name, spec):
        row = dve_ops._CUSTOM_DVE_ROW_BASE + len(dve_ops.OPS)
        dve_ops._SUB_OPCODE_FOR_NAME[name] = row
        shas = {}
        for ver in ("v3", "v4"):
            try:
                uops = lower(spec, ver=ver)
                shas[ver] = DveOpSpec(
                    name=name, opcode=row, uops=uops,
                    rd1_en=_has_src1(spec)).sha(ver)
            except Exception:
                pass
        op = dve_ops.DveOp(name, spec, subdim=False, uops_sha=shas)
        dve_ops.OPS.append(op)
        dve_ops.CUSTOM_DVE_SPECS[name] = spec
        return op

    body1 = (maxx(Src0 + C0, C1) - (C1 - One)) * C2

    def ref1(in0, in1, s0, s1, imm2):
        zc = (in0.astype(np.float32) + np.float32(s0)).astype(np.float32)
        zcc = np.maximum(zc, np.float32(s1))
        return ((zcc - (np.float32(s1) - np.float32(1.0))) *
                np.float32(imm2)).astype(np.float32)

    op1 = mk("EXP2K_HEPT", Spec(body=body1, reference=ref1))

    zc = Src0 + C0
    fc = (Src0 + (C0 - zc)) + C1
    body2 = ((sq(fc) + C1 * C1) * C2) * Src1

    def ref2(in0, in1, s0, s1, imm2):
        z = (in0.astype(np.float32) + np.float32(s0)).astype(np.float32)
        f0 = (in0.astype(np.float32) +
              (np.float32(s0) - z)).astype(np.float32)
        fc_ = (f0 + np.float32(s1)).astype(np.float32)
        return (((fc_ * fc_ + np.float32(s1) * np.float32(s1)) *
                 np.float32(imm2)) * in1).astype(np.float32)

    op2 = mk("EXP2P_HEPT", Spec(body=body2, reference=ref2))
    return op1, op2


def _register_expbits():
    """One-pass exp: out_u16 = bits_bf16(2^P2) for Src0 = G = 128*P2+16320.

    w = round_128(Src0) = 128*(K+128); x = Src0-w = 128*frac-64;
    out = beta*(x+C1)^2 + w + (kappa-128), stored with a saturating
    f32->uint16 convert (deep-underflow negatives become 0)."""
    from concourse import dve_ops
    from concourse.dve_spec import (
        Spec, Src0, Src1, C0, C1, C2, lower, sq, _has_src1,
    )
    from concourse.dve_uop import DveOpSpec

    if "EXPBITS_HEPT" in dve_ops._SUB_OPCODE_FOR_NAME:
        return {op.name: op for op in dve_ops.OPS}["EXPBITS_HEPT"]

    w = (Src0 + C0) - C0
    x = Src0 - w
    u = x + C1
    body = ((sq(u) * C2) + w) + Src1

    def ref(in0, in1, s0, s1, imm2):
        f32 = np.float32
        z = (in0.astype(f32) + f32(s0)).astype(f32)
        wv = (z - f32(s0)).astype(f32)
        xv = (in0.astype(f32) - wv).astype(f32)
        uv = (xv + f32(s1)).astype(f32)
        return (((uv * uv).astype(f32) * f32(imm2) + wv)
                + in1.astype(f32)).astype(f32)

    spec = Spec(body=body, reference=ref)
    row = dve_ops._CUSTOM_DVE_ROW_BASE + len(dve_ops.OPS)
    dve_ops._SUB_OPCODE_FOR_NAME["EXPBITS_HEPT"] = row
    shas = {}
    for ver in ("v3", "v4"):
        try:
            uops = lower(spec, ver=ver)
            shas[ver] = DveOpSpec(name="EXPBITS_HEPT", opcode=row, uops=uops,
                                  rd1_en=_has_src1(spec)).sha(ver)
        except Exception:
            pass
    op = dve_ops.DveOp("EXPBITS_HEPT", spec, subdim=False, uops_sha=shas)
    dve_ops.OPS.append(op)
    dve_ops.CUSTOM_DVE_SPECS["EXPBITS_HEPT"] = spec
    return op


def build_nc_split1p(n_quads=NQUAD, ipool_bufs=8, apool_bufs=3, opool_bufs=4):
    """Per-group column split: ACT exps 12 blocks (cols 0:1536, bf16 out),
    the one-pass DVE bits-exp handles 4 blocks (cols 1536:2048, uint16 out
    bitcast to bf16); mm2 all-bf16; DVE-half mm2+cast emitted first so the
    PSUM recycle chain ends on the smaller ACT-half cast; output DMA on the
    (otherwise idle) gpsimd queue.  Output block order within a group is
    lane-major (l*4+gq) -- host unpermutes."""
    import concourse.mybir as mybir
    import concourse.tile as tile
    from concourse.bacc import Bacc

    op = _register_expbits()
    f16 = mybir.dt.float16
    bf16 = mybir.dt.bfloat16
    f32 = mybir.dt.float32
    u16 = mybir.dt.uint16
    nblk = n_quads * 4
    ngroups = nblk // 16

    nc = Bacc()
    im = nc.declare_dram_parameter("in", [128, ngroups * GRP_COLS], f16,
                                   isOutput=False)
    om = nc.declare_dram_parameter("out", [128, nblk * 24], f16, isOutput=True)

    with tile.TileContext(nc) as tc, ExitStack() as ctx:
        ipool = ctx.enter_context(tc.tile_pool(name="ipool", bufs=ipool_bufs))
        apool = ctx.enter_context(tc.tile_pool(name="apool", bufs=apool_bufs))
        abpool = ctx.enter_context(tc.tile_pool(name="abpool", bufs=apool_bufs))
        opool = ctx.enter_context(tc.tile_pool(name="opool", bufs=opool_bufs))
        dpool = ctx.enter_context(tc.tile_pool(name="dummy", bufs=1))
        ps1pool = ctx.enter_context(tc.tile_pool(name="ps1", bufs=2, space="PSUM"))

        # constants + ACT exp-table warm while the first input DMAs fly
        bias_t = dpool.tile([128, 1], f32)
        nc.vector.memset(bias_t, -LN2 * 127.5)
        kap = dpool.tile([128, DVE_COLS], f32)
        nc.vector.memset(kap, SRC1_K)
        dmy = dpool.tile([128, 8], f32)
        nc.vector.memset(dmy, 0.0)
        dmy2 = dpool.tile([128, 8], bf16)
        nc.scalar.activation(out=dmy2, in_=dmy,
                             func=mybir.ActivationFunctionType.Exp,
                             scale=LN2 / 128.0, bias=bias_t[:, 0:1])

        PREFETCH = ipool_bufs - 2
        tiles = {}

        def fetch(g):
            if g >= ngroups or g in tiles:
                return
            in_t = ipool.tile([128, GRP_COLS], f16)
            nc.sync.dma_start(out=in_t, in_=im[:, g * GRP_COLS:(g + 1) * GRP_COLS])
            tiles[g] = in_t

        NA = ACT_COLS // 128           # blocks exp'd on ACT (device cols first)

        def consume(st):
            """Device out position p = col//128 (lane-major); natural block
            id there is gq*4+l.  DVE-half mm2s + their cast go first so the
            PSUM recycle chain ends on the smaller ACT-half cast."""
            ps1, a_t, ab_t, in_t, g = st
            v_t = in_t[:, 2 * GRP_Q:GRP_COLS].bitcast(bf16)
            o_t = opool.tile([128, 16 * 24], f16)

            def mm2(p):
                col = p * 128
                l, gq = col // 512, (col % 512) // 128
                if col >= ACT_COLS:
                    lhs = ab_t[:, col - ACT_COLS:col - ACT_COLS + 128].bitcast(bf16)
                else:
                    lhs = a_t[:, col:col + 128]
                nc.tensor.matmul(
                    out=ps1[:, p * 24:(p + 1) * 24], lhsT=lhs,
                    rhs=v_t[:, (gq * 4 + l) * 24:(gq * 4 + l + 1) * 24],
                    start=True, stop=True,
                )

            for p in range(NA, 16):        # DVE-half first
                mm2(p)
            nc.vector.tensor_copy(out=o_t[:, NA * 24:384], in_=ps1[:, NA * 24:384])
            for p in range(NA):
                mm2(p)
            nc.vector.tensor_copy(out=o_t[:, 0:NA * 24], in_=ps1[:, 0:NA * 24])
            nc.gpsimd.dma_start(out=om[:, g * 384:(g + 1) * 384], in_=o_t)

        fetch(0)
        fetch(1)
        fetched = [2]

        def topup(g):
            n = 0
            while (fetched[0] < ngroups and fetched[0] <= g + PREFETCH
                   and n < 2):
                fetch(fetched[0])
                fetched[0] += 1
                n += 1

        pend = None
        for g in range(ngroups):
            in_t = tiles.pop(g)
            q_t = in_t[:, 0:GRP_Q]
            k_t = in_t[:, GRP_Q:2 * GRP_Q]
            ps1 = ps1pool.tile([128, 2048], f32)
            for gq in range(4):
                for l in range(4):
                    col = l * 512 + gq * 128
                    nc.tensor.matmul(
                        out=ps1[:, col:col + 128],
                        lhsT=k_t[32 * l:32 * l + 32, gq * 128:(gq + 1) * 128],
                        rhs=q_t[32 * l:32 * l + 32, gq * 128:(gq + 1) * 128],
                        start=True, stop=True,
                        tile_position=(32 * l, 0),
                    )
            topup(g)
            a_t = apool.tile([128, ACT_COLS], bf16)
            nc.scalar.activation(
                out=a_t, in_=ps1[:, 0:ACT_COLS],
                func=mybir.ActivationFunctionType.Exp,
                scale=LN2 / 128.0, bias=bias_t[:, 0:1],
            )
            ab_t = abpool.tile([128, DVE_COLS], u16)
            nc.vector._custom_dve(
                op, out=ab_t, in0=ps1[:, ACT_COLS:2048], in1=kap,
                s0=C0_MAGIC, s1=C1_VERT, imm2=BETA_1P,
            )
            if pend is not None:
                consume(pend)
            pend = (ps1, a_t, ab_t, in_t, g)
        consume(pend)
    nc.finalize()
    return nc


def build_nc(n_quads=NQUAD, ipool_bufs=3, apool_bufs=3, opool_bufs=3):
    """Build the per-core Bass module (same NEFF for all 8 cores).

    Constraints baked in:
    - One combined input DMA per super-tile (q|k|v).
    - Row-tiled (tile_position) matmuls sharing a PSUM *bank* crash the
      device -> the 4 lanes of a quad write 4 distinct banks: lanes 0-2
      into ps_act (3 banks), lane 3 into ps_dve (1 bank).
    - Every accessor of one PSUM tile is serialised by the framework,
      so ACT work and DVE work live on separate PSUM tiles.
    """
    import concourse.mybir as mybir
    import concourse.tile as tile
    from concourse.bacc import Bacc

    op1, op2 = _register_exp_ops()

    f16 = mybir.dt.float16
    f32 = mybir.dt.float32
    i32 = mybir.dt.int32
    nblk = n_quads * 4
    assert n_quads % SUPER_Q == 0
    n_super = n_quads // SUPER_Q

    nc = Bacc()
    im = nc.declare_dram_parameter("in", [128, n_super * SUP_COLS], f16,
                                   isOutput=False)
    om = nc.declare_dram_parameter("out", [128, nblk * 24], f16, isOutput=True)

    with tile.TileContext(nc) as tc, ExitStack() as ctx:
        ipool = ctx.enter_context(tc.tile_pool(name="ipool", bufs=ipool_bufs))
        apool = ctx.enter_context(tc.tile_pool(name="apool", bufs=apool_bufs))
        adpool = ctx.enter_context(tc.tile_pool(name="adpool", bufs=apool_bufs))
        ypool = ctx.enter_context(tc.tile_pool(name="ypool", bufs=apool_bufs))
        opool = ctx.enter_context(tc.tile_pool(name="opool", bufs=opool_bufs))
        pa_pool = ctx.enter_context(tc.tile_pool(name="psA", bufs=2, space="PSUM"))
        pd_pool = ctx.enter_context(tc.tile_pool(name="psD", bufs=2, space="PSUM"))

        def emit_mm2_and_drain(st):
            """Consume stage st = (ps_a, ps_d, aa, ad, v_t, og_base, b0):
            mm2 for all 16 blocks, then cast + DMA the outputs."""
            ps_a, ps_d, aa, ad, v_t, og4, b0 = st
            for bb in range(16):
                if bb < NACT_BLK:
                    gq, l = bb // 3, bb % 3
                    lhs = aa[:, (l * 512 + gq * 128):(l * 512 + gq * 128 + 128)]
                    out_ap = ps_a[:, bb * 24:bb * 24 + 24]
                else:
                    gq, l = bb - NACT_BLK, 3
                    lhs = ad[:, gq * 128:gq * 128 + 128]
                    out_ap = ps_d[:, (bb - NACT_BLK) * 24:
                                  (bb - NACT_BLK) * 24 + 24]
                b_local = (og4 + gq) * 4 + l
                nc.tensor.matmul(
                    out=out_ap,
                    lhsT=lhs,
                    rhs=v_t[:, b_local * 24:(b_local + 1) * 24],
                    start=True, stop=True,
                )
            o_t = opool.tile([128, 16 * 24], f16)
            if CAST_ACT > 0:
                nc.scalar.copy(
                    out=o_t[:, 0:CAST_ACT], in_=ps_a[:, 0:CAST_ACT])
                nc.vector.tensor_copy(
                    out=o_t[:, CAST_ACT:288], in_=ps_a[:, CAST_ACT:288])
            else:
                nc.vector.tensor_copy(out=o_t[:, 0:288], in_=ps_a[:, 0:288])
            nc.vector.tensor_copy(out=o_t[:, 288:384], in_=ps_d[:, 0:96])
            nc.sync.dma_start(out=om[:, b0 * 24:(b0 + 16) * 24], in_=o_t)

        pend = None                             # previous group's stage
        for s in range(n_super):
            in_t = ipool.tile([128, SUP_COLS], f16)
            c0 = s * SUP_COLS
            nc.sync.dma_start(out=in_t, in_=im[:, c0:c0 + SUP_COLS])
            q_t = in_t[:, 0:SUP_Q_COLS]
            k_t = in_t[:, SUP_Q_COLS:2 * SUP_Q_COLS]
            v_t = in_t[:, 2 * SUP_Q_COLS:SUP_COLS]

            for og in range(4):                 # 4 groups of 4 quads (16 blk)
                ps_a = pa_pool.tile([128, 1536], f32)   # banks for lanes 0-2
                ps_d = pd_pool.tile([128, 512], f32)    # bank for lane 3
                for gq in range(4):             # ACT lanes first
                    qd = og * 4 + gq
                    for l in range(3):
                        col = l * 512 + gq * 128
                        nc.tensor.matmul(
                            out=ps_a[:, col:col + 128],
                            lhsT=k_t[32 * l:32 * l + 32, qd * 128:(qd + 1) * 128],
                            rhs=q_t[32 * l:32 * l + 32, qd * 128:(qd + 1) * 128],
                            start=True, stop=True,
                            tile_position=(32 * l, 0),
                        )
                for gq in range(4):             # DVE lane last
                    qd = og * 4 + gq
                    nc.tensor.matmul(
                        out=ps_d[:, gq * 128:gq * 128 + 128],
                        lhsT=k_t[96:128, qd * 128:(qd + 1) * 128],
                        rhs=q_t[96:128, qd * 128:(qd + 1) * 128],
                        start=True, stop=True,
                        tile_position=(96, 0),
                    )
                aa = apool.tile([128, 1536], f16)
                ad = adpool.tile([128, 512], f16)
                y_t = ypool.tile([128, 512], f32)
                nc.scalar.activation(
                    out=aa, in_=ps_a,
                    func=mybir.ActivationFunctionType.Exp,
                    scale=LN2,
                )
                nc.vector._custom_dve(
                    op1, out=y_t.bitcast(i32), in0=ps_d,
                    s0=DVE_M, s1=DVE_CLO, imm2=DVE_E23,
                )
                nc.vector._custom_dve(
                    op2, out=ad, in0=ps_d,
                    in1=y_t, s0=DVE_M, s1=DVE_C, imm2=DVE_A,
                )
                if pend is not None:            # consume the PREVIOUS group
                    emit_mm2_and_drain(pend)
                pend = (ps_a, ps_d, aa, ad, v_t, og * 4, (s * 4 + og) * 16)
        emit_mm2_and_drain(pend)
    nc.finalize()
    return nc


def build_nc_allact(n_quads=NQUAD, ipool_bufs=8, apool_bufs=3, opool_bufs=3):
    """Single-PSUM-pool variant: ALL exp on ScalarE (one 2048-col ACT
    instr per group keeps the scalar queue saturated), casts on VectorE,
    mm2 + drain software-pipelined one group behind the fills so every
    wait is pre-posted when its consumer reaches the queue head.
    Input arrives in per-group chunks (1408 cols) DMA'd several groups
    ahead so super-boundary stalls disappear."""
    import concourse.mybir as mybir
    import concourse.tile as tile
    from concourse.bacc import Bacc

    f16 = mybir.dt.float16
    f32 = mybir.dt.float32
    nblk = n_quads * 4
    ngroups = nblk // 16

    nc = Bacc()
    im = nc.declare_dram_parameter("in", [128, ngroups * GRP_COLS], f16,
                                   isOutput=False)
    om = nc.declare_dram_parameter("out", [128, nblk * 24], f16, isOutput=True)

    with tile.TileContext(nc) as tc, ExitStack() as ctx:
        ipool = ctx.enter_context(tc.tile_pool(name="ipool", bufs=ipool_bufs))
        apool = ctx.enter_context(tc.tile_pool(name="apool", bufs=apool_bufs))
        opool = ctx.enter_context(tc.tile_pool(name="opool", bufs=opool_bufs))
        dpool = ctx.enter_context(tc.tile_pool(name="dummy", bufs=1))
        ps1pool = ctx.enter_context(tc.tile_pool(name="ps1", bufs=2, space="PSUM"))

        # warm the exp ACT table while the first input DMAs are in flight
        dmy = dpool.tile([128, 8], f32)
        nc.vector.memset(dmy, 0.0)
        dmy2 = dpool.tile([128, 8], f16)
        nc.scalar.activation(out=dmy2, in_=dmy,
                             func=mybir.ActivationFunctionType.Exp)

        PREFETCH = ipool_bufs - 2
        tiles = {}

        def fetch(g):
            if g >= ngroups or g in tiles:
                return
            in_t = ipool.tile([128, GRP_COLS], f16)
            nc.sync.dma_start(out=in_t, in_=im[:, g * GRP_COLS:(g + 1) * GRP_COLS])
            tiles[g] = in_t

        def consume(st):
            ps1, a_t, in_t, b0 = st
            v_t = in_t[:, 2 * GRP_Q:GRP_COLS]
            for bb in range(16):
                gq, l = bb // 4, bb % 4
                acol = l * 512 + gq * 128
                nc.tensor.matmul(
                    out=ps1[:, bb * 24:(bb + 1) * 24],
                    lhsT=a_t[:, acol:acol + 128],
                    rhs=v_t[:, (gq * 4 + l) * 24:(gq * 4 + l + 1) * 24],
                    start=True, stop=True,
                )
            o_t = opool.tile([128, 16 * 24], f16)
            nc.vector.tensor_copy(out=o_t, in_=ps1[:, 0:384])
            nc.sync.dma_start(out=om[:, b0 * 24:(b0 + 16) * 24], in_=o_t)

        # Stagger the prefetch buildup: only groups 0-1 before the loop so
        # group 0's chunk isn't queued behind 2MB of lookahead traffic;
        # depth then grows by up to 2 fetches per group until PREFETCH.
        fetch(0)
        fetch(1)
        fetched = [2]

        def topup(g):
            n = 0
            while (fetched[0] < ngroups and fetched[0] <= g + PREFETCH
                   and n < 2):
                fetch(fetched[0])
                fetched[0] += 1
                n += 1

        pend = None
        for g in range(ngroups):
            in_t = tiles.pop(g)
            q_t = in_t[:, 0:GRP_Q]
            k_t = in_t[:, GRP_Q:2 * GRP_Q]
            ps1 = ps1pool.tile([128, 2048], f32)
            for gq in range(4):
                for l in range(4):
                    col = l * 512 + gq * 128
                    nc.tensor.matmul(
                        out=ps1[:, col:col + 128],
                        lhsT=k_t[32 * l:32 * l + 32, gq * 128:(gq + 1) * 128],
                        rhs=q_t[32 * l:32 * l + 32, gq * 128:(gq + 1) * 128],
                        start=True, stop=True,
                        tile_position=(32 * l, 0),
                    )
            topup(g)
            a_t = apool.tile([128, 2048], f16)
            nc.scalar.activation(
                out=a_t, in_=ps1,
                func=mybir.ActivationFunctionType.Exp,
                scale=LN2,
            )
            if pend is not None:
                consume(pend)
            pend = (ps1, a_t, in_t, g * 16)
        consume(pend)
    nc.finalize()
    return nc


def build_nc_split2(n_quads=NQUAD, ipool_bufs=8, apool_bufs=3, opool_bufs=3):
    """Per-group-DMA + software-pipelined variant with the exp split
    across ScalarE (lanes 0-2, 3 PSUM banks) and VectorE custom-DVE
    (lane 3, 1 PSUM bank).  Output casts: ps_a's 288 cols on ACT (its
    own pool), ps_d's 96 cols on DVE."""
    import concourse.mybir as mybir
    import concourse.tile as tile
    from concourse.bacc import Bacc

    op1, op2 = _register_exp_ops()
    f16 = mybir.dt.float16
    f32 = mybir.dt.float32
    i32 = mybir.dt.int32
    nblk = n_quads * 4
    ngroups = nblk // 16

    nc = Bacc()
    im = nc.declare_dram_parameter("in", [128, ngroups * GRP_COLS], f16,
                                   isOutput=False)
    om = nc.declare_dram_parameter("out", [128, nblk * 24], f16, isOutput=True)

    with tile.TileContext(nc) as tc, ExitStack() as ctx:
        ipool = ctx.enter_context(tc.tile_pool(name="ipool", bufs=ipool_bufs))
        apool = ctx.enter_context(tc.tile_pool(name="apool", bufs=apool_bufs))
        adpool = ctx.enter_context(tc.tile_pool(name="adpool", bufs=apool_bufs))
        ypool = ctx.enter_context(tc.tile_pool(name="ypool", bufs=apool_bufs))
        opool = ctx.enter_context(tc.tile_pool(name="opool", bufs=opool_bufs))
        dpool = ctx.enter_context(tc.tile_pool(name="dummy", bufs=1))
        pa_pool = ctx.enter_context(tc.tile_pool(name="psA", bufs=2, space="PSUM"))
        pd_pool = ctx.enter_context(tc.tile_pool(name="psD", bufs=2, space="PSUM"))

        # warm the exp ACT table while the first input DMA is in flight
        dmy = dpool.tile([128, 8], f32)
        nc.vector.memset(dmy, 0.0)
        dmy2 = dpool.tile([128, 8], f16)
        nc.scalar.activation(out=dmy2, in_=dmy,
                             func=mybir.ActivationFunctionType.Exp)

        PREFETCH = ipool_bufs - 2
        tiles = {}

        def fetch(g):
            if g >= ngroups or g in tiles:
                return
            in_t = ipool.tile([128, GRP_COLS], f16)
            nc.sync.dma_start(out=in_t, in_=im[:, g * GRP_COLS:(g + 1) * GRP_COLS])
            tiles[g] = in_t

        def consume(st):
            ps_a, ps_d, aa, ad, in_t, b0 = st
            v_t = in_t[:, 2 * GRP_Q:GRP_COLS]
            for bb in range(16):
                if bb < NACT_BLK:
                    gq, l = bb // 3, bb % 3
                    lhs = aa[:, (l * 512 + gq * 128):(l * 512 + gq * 128 + 128)]
                    out_ap = ps_a[:, bb * 24:bb * 24 + 24]
                else:
                    gq, l = bb - NACT_BLK, 3
                    lhs = ad[:, gq * 128:gq * 128 + 128]
                    out_ap = ps_d[:, (bb - NACT_BLK) * 24:
                                  (bb - NACT_BLK) * 24 + 24]
                nc.tensor.matmul(
                    out=out_ap,
                    lhsT=lhs,
                    rhs=v_t[:, (gq * 4 + l) * 24:(gq * 4 + l + 1) * 24],
                    start=True, stop=True,
                )
            o_t = opool.tile([128, 16 * 24], f16)
            nc.scalar.copy(out=o_t[:, 0:96], in_=ps_a[:, 0:96])
            nc.vector.tensor_copy(out=o_t[:, 96:288], in_=ps_a[:, 96:288])
            nc.vector.tensor_copy(out=o_t[:, 288:384], in_=ps_d[:, 0:96])
            nc.sync.dma_start(out=om[:, b0 * 24:(b0 + 16) * 24], in_=o_t)

        for g in range(PREFETCH):
            fetch(g)
        pend = None
        for g in range(ngroups):
            in_t = tiles.pop(g)
            q_t = in_t[:, 0:GRP_Q]
            k_t = in_t[:, GRP_Q:2 * GRP_Q]
            ps_a = pa_pool.tile([128, 1536], f32)
            ps_d = pd_pool.tile([128, 512], f32)
            for gq in range(4):
                for l in range(3):
                    col = l * 512 + gq * 128
                    nc.tensor.matmul(
                        out=ps_a[:, col:col + 128],
                        lhsT=k_t[32 * l:32 * l + 32, gq * 128:(gq + 1) * 128],
                        rhs=q_t[32 * l:32 * l + 32, gq * 128:(gq + 1) * 128],
                        start=True, stop=True,
                        tile_position=(32 * l, 0),
                    )
            for gq in range(4):
                nc.tensor.matmul(
                    out=ps_d[:, gq * 128:gq * 128 + 128],
                    lhsT=k_t[96:128, gq * 128:(gq + 1) * 128],
                    rhs=q_t[96:128, gq * 128:(gq + 1) * 128],
                    start=True, stop=True,
                    tile_position=(96, 0),
                )
            fetch(g + PREFETCH)
            aa = apool.tile([128, 1536], f16)
            ad = adpool.tile([128, 512], f16)
            y_t = ypool.tile([128, 512], f32)
            nc.scalar.activation(
                out=aa, in_=ps_a,
                func=mybir.ActivationFunctionType.Exp,
                scale=LN2,
            )
            nc.vector._custom_dve(
                op1, out=y_t.bitcast(i32), in0=ps_d,
                s0=DVE_M, s1=DVE_CLO, imm2=DVE_E23,
            )
            nc.vector._custom_dve(
                op2, out=ad, in0=ps_d,
                in1=y_t, s0=DVE_M, s1=DVE_C, imm2=DVE_A,
            )
            if pend is not None:
                consume(pend)
            pend = (ps_a, ps_d, aa, ad, in_t, g * 16)
        consume(pend)
    nc.finalize()
    return nc


def _get_nc(n_quads=NQUAD):
    key = (n_quads, MODE)
    if key not in _NC_CACHE:
        if MODE == "split1p":
            _NC_CACHE[key] = build_nc_split1p(n_quads)
        elif MODE == "allact":
            _NC_CACHE[key] = build_nc_allact(n_quads)
        elif MODE == "split2":
            _NC_CACHE[key] = build_nc_split2(n_quads)
        else:
            _NC_CACHE[key] = build_nc(n_quads)
    return _NC_CACHE[key]


# ---------------- host-side preparation ----------------

def _sort_indices(query, key, combined_shifts, alpha):
    """Replicate the reference's hash + argsort with jax on CPU.

    Uses the exact same jnp ops the reference uses so the fp32 values
    (and therefore the argsort permutations) match bit-for-bit.
    """
    import jax
    import jax.numpy as jnp

    cpu = jax.devices("cpu")[0]
    with jax.default_device(cpu):
        q = jnp.asarray(query)
        k = jnp.asarray(key)
        al = jnp.asarray(alpha)
        cs_i = jnp.asarray(combined_shifts)
        q_hashed = jnp.einsum('hnd,hdr->rhn', q, al)
        k_hashed = jnp.einsum('hnd,hdr->rhn', k, al)
        max_shift = jnp.maximum(q_hashed.max(-1, keepdims=True),
                                k_hashed.max(-1, keepdims=True))
        min_shift = jnp.minimum(q_hashed.min(-1, keepdims=True),
                                k_hashed.min(-1, keepdims=True))
        hash_shift = max_shift - min_shift
        cs = cs_i.astype(q_hashed.dtype) * hash_shift
        q_pos = np.asarray(jnp.argsort(q_hashed + cs, axis=-1))
        k_pos = np.asarray(jnp.argsort(k_hashed + cs, axis=-1))
    return q_pos, k_pos


def _split16(x):
    hi = x.astype(np.float16)
    lo = (x - hi.astype(np.float32)).astype(np.float16)
    return hi, lo


def _build_stack(s_qk, is_k, scale=SQRT_LOG2E, offset=0.0):
    """(UNITS*NB, 128, 27) f32 -> (UNITS*NB, 32, 128) fp16 stack.

    Data rows are pre-scaled by `scale` so the on-device Gram is
    scale^2 * (-0.5*||q-k||^2); the norm rows use the scaled fp16 data
    and carry `offset` (split1p bakes +8160 per side into the Gram).
    """
    nblk = s_qk.shape[0]
    hi = (s_qk * np.float32(scale)).astype(np.float16)  # (b, i, d)
    sqm = -0.5 * np.einsum('bid,bid->bi', hi.astype(np.float32),
                           hi.astype(np.float32)) + np.float32(offset)
    sq_hi, sq_lo = _split16(sqm)
    st = np.zeros((nblk, KROWS, BLOCK), np.float16)
    st[:, :D_QK, :] = hi.transpose(0, 2, 1)            # rows 0-26: x^T
    if is_k:
        st[:, 27, :] = 1.0                             # pair of q's sq rows
        st[:, 28, :] = 1.0
        st[:, 29, :] = sq_hi                           # -0.5*||k||^2 hi
        st[:, 30, :] = sq_lo
    else:
        st[:, 27, :] = sq_hi                           # -0.5*||q||^2 hi
        st[:, 28, :] = sq_lo
        st[:, 29, :] = 1.0                             # pair of k's sq rows
        st[:, 30, :] = 1.0
    return st


def _pack_core(stack_blocks):
    """(768, 32, 128) -> (128, 192*128): partition = lane*32+row,
    free = quad*128 + col."""
    return (stack_blocks.reshape(NQUAD, 4, KROWS, BLOCK)
            .transpose(1, 2, 0, 3)
            .reshape(128, NQUAD * BLOCK))


# om block order within each 16-block group: 12 ACT blocks (lanes 0-2 in
# (quad, lane) order) then 4 DVE blocks (lane 3).  _OM_PERM[bb] = b_local
# offset within the group's 16 blocks.
_OM_PERM = np.array(
    [q * 4 + l for q in range(4) for l in range(3)] +
    [q * 4 + 3 for q in range(4)], dtype=np.int64)


def prepare_in_maps(query, key, value, combined_shifts, alpha):
    query = np.ascontiguousarray(np.asarray(query), dtype=np.float32)
    key = np.ascontiguousarray(np.asarray(key), dtype=np.float32)
    value = np.ascontiguousarray(np.asarray(value), dtype=np.float32)
    combined_shifts = np.asarray(combined_shifts)
    alpha = np.asarray(alpha, dtype=np.float32)

    q_pos, k_pos = _sort_indices(query, key, combined_shifts, alpha)

    h_idx = np.arange(N_HEADS)[None, :, None]
    s_query = query[h_idx, q_pos].reshape(UNITS * NB, BLOCK, D_QK)
    s_key = key[h_idx, k_pos].reshape(UNITS * NB, BLOCK, D_QK)
    s_value = value[h_idx, k_pos].reshape(UNITS * NB, BLOCK, DIM_PER_HEAD)

    if MODE == "split1p":
        qstack = _build_stack(s_query, is_k=False, scale=SC2, offset=G_OFS / 2)
        kstack = _build_stack(s_key, is_k=True, scale=SC2, offset=G_OFS / 2)
        # v as bf16 bit patterns carried in the f16-typed input buffer
        vb = np.ascontiguousarray(s_value, np.float32).view(np.uint32)
        vb = ((vb + 0x7FFF + ((vb >> 16) & 1)) >> 16).astype(np.uint16)
        v16 = vb.view(np.float16)
    else:
        qstack = _build_stack(s_query, is_k=False)
        kstack = _build_stack(s_key, is_k=True)
        v16 = s_value.astype(np.float16)

    in_maps = []
    for c in range(N_CORES):
        b0, b1 = c * NBLK, (c + 1) * NBLK
        qp = _pack_core(qstack[b0:b1])              # [128, NQUAD*128]
        kp = _pack_core(kstack[b0:b1])
        vp = v16[b0:b1].transpose(1, 0, 2).reshape(128, NBLK * 24)
        combined = np.empty((128, NGROUPS * GRP_COLS), np.float16)
        for g in range(NGROUPS):
            c0 = g * GRP_COLS
            combined[:, c0:c0 + GRP_Q] = qp[:, g * GRP_Q:(g + 1) * GRP_Q]
            combined[:, c0 + GRP_Q:c0 + 2 * GRP_Q] = \
                kp[:, g * GRP_Q:(g + 1) * GRP_Q]
            combined[:, c0 + 2 * GRP_Q:c0 + GRP_COLS] = \
                vp[:, g * GRP_V:(g + 1) * GRP_V]
        in_maps.append({"in": combined})
    return in_maps


def assemble_output(results):
    """results: list of 8 dicts with 'out' [128, 768*24] f16 in the
    permuted (ACT-blocks-first) group order."""
    ngroups = NBLK // 16
    if MODE == "allact":
        om_perm = np.arange(16)
    elif MODE == "split1p":
        # device col p = l*4+gq holds natural block gq*4+l
        om_perm = np.array([(p % 4) * 4 + p // 4 for p in range(16)],
                          dtype=np.int64)
    else:
        om_perm = _OM_PERM
    perm = (np.arange(ngroups)[:, None] * 16 + om_perm[None, :]).ravel()
    inv = np.empty_like(perm)
    inv[perm] = np.arange(NBLK)
    out = np.empty((UNITS, NB, BLOCK, DIM_PER_HEAD), np.float32)
    for c in range(N_CORES):
        so = np.asarray(results[c]["out"]).astype(np.float32)
        so = so.reshape(128, NBLK, 24)[:, inv, :]
        out[c * UPC:(c + 1) * UPC] = (
            so.transpose(1, 0, 2).reshape(UPC, NB, BLOCK, DIM_PER_HEAD))
    return out.reshape(N_HASHES, N_HEADS, NB, BLOCK, DIM_PER_HEAD)


def run(query, key, value, combined_shifts, alpha, trace=False):
    from concourse.bass_utils import run_bass_kernel_spmd

    in_maps = prepare_in_maps(query, key, value, combined_shifts, alpha)
    nc = _get_nc()
    res = run_bass_kernel_spmd(
        nc, in_maps, core_ids=list(range(N_CORES)), trace=trace)
    out = assemble_output(res.results)
    return out, res


def kernel(query, key, value, combined_shifts, alpha):
    out, _ = run(query, key, value, combined_shifts, alpha,
                 trace=bool(int(os.environ.get("HEPT_TRACE", "0"))))
    return out



# revision 12
# speedup vs baseline: 1.3813x; 1.3813x over previous
"""HEPT sparse attention for Trainium2 — 8-core SPMD Bass kernel.

Reference computation (per hash-round r, head h):
  hash q/k via shared projection, argsort, gather into blocks of 128,
  blocked RBF attention: so = exp(-0.5*||q_i-k_j||^2) @ v.

Strategy (MODE="allact", the measured-fastest variant):
  - Host: bitwise-exact hash + argsort (jax CPU, identical ops to the
    reference), gather, fp16 quantization (data rows pre-scaled by
    sqrt(log2e) so the device Gram is in log2 units), per-group layout
    packing (q|k|v chunks of 1408 cols per 16-block group).
  - Device (per core, 3 of the 24 (r,h) units = 768 blocks): the whole
    log2e*-0.5*||q-k||^2 Gram of one block is ONE K=32 fp16 matmul
    (rows 0-26 data, rows 27-30 squared-norm terms hi/lo paired with
    ones rows); 4 blocks share the PE via row tiling into 4 PSUM banks.
    exp runs as one 2048-col ScalarE ACT instruction per group
    (exp(ln2*P) = 2^P); mm2 (so = A @ v, A fp16 weights) writes back
    into exp-consumed PSUM; VectorE casts f32->f16 for the output DMA.
  - Pipeline: per-group input DMAs prefetched 6 groups ahead, the exp
    ACT table preloaded during the DMA prefill, and mm2 + cast + output
    DMA emitted one group behind the fills, so every wait is pre-posted
    and ScalarE (the bottleneck engine, ~94 us busy) runs back-to-back.
  - A "split2" variant (custom-DVE 2-pass exp for a quarter of the
    Gram on VectorE) measured consistently slower (~127-132 us vs
    ~123 us) due to cross-engine handoff latency on the extra PSUM
    recycle chains, and is kept for reference.
"""

import math
import os
from contextlib import ExitStack

import numpy as np

# ---- problem constants (hardcoded; kernel.py must be self-contained) ----
N_HASHES = 3
N_HEADS = 8
PADDED_SIZE = 32768
BLOCK = 128
DIM_PER_HEAD = 24
D_QK = 27
NB = PADDED_SIZE // BLOCK          # 256 blocks per (r,h)
N_CORES = 8
UNITS = N_HASHES * N_HEADS         # 24 independent (r,h) units
UPC = UNITS // N_CORES             # 3 units per core
NBLK = UPC * NB                    # 768 blocks per core
NQUAD = NBLK // 4                  # 192 quads per core (4 blocks/quad)
SUPER_Q = 16                       # quads per super-tile (64 blocks)
KROWS = 32                         # stacked contraction rows per block

LOG2E = 1.4426950408889634
SQRT_LOG2E = math.sqrt(LOG2E)
LN2 = 0.6931471805599453

# custom-DVE exp constants
DVE_M = 12582912.0                 # 1.5 * 2^23 round-to-int magic
DVE_CLO = DVE_M - 126.0            # clamp: K >= -126
DVE_E23 = 8388608.0                # 2^23
DVE_A = 0.24973141119916378        # minimax A*((s+c)^2 + c^2) ~ 2^s, |s|<=.5
DVE_C = 1.414837949227267

NACT_BLK = 12                      # blocks/group exp'd on ACT (lanes 0-2)
NDVE_BLK = 4                       # blocks/group exp'd on DVE (lane 3)
CAST_ACT = 0                       # output cols cast on ACT (0 = all on DVE)
MODE = os.environ.get("HEPT_MODE", "split1p")

# ---- split1p mode: one-pass DVE bits-exp + ACT exp column split ----
# Gram G = 128*log2(e)*(-0.5*||q-k||^2) + 16320 (offset baked via norm rows).
# ACT path: exp(G*ln2/128 - 127.5*ln2) = 2^P2, written bf16.
# DVE path: one 8-slice custom op emits the uint16 BIT PATTERN of the bf16
#   value 2^P2 (magic-add exponent extraction + minimax quadratic mantissa),
#   negative (underflow) results clamp to 0 via the saturating uint16 store.
SC2 = math.sqrt(128.0 * LOG2E)     # host pre-scale for q/k data rows
G_OFS = 16320.0                    # = 128*127.5, split 8160 per side
BETA_1P = 0.00268750865            # minimax quad: beta*(y+H)^2 + kappa
H_1P = 121.124593
KAPPA_1P = -39.1119528
C0_MAGIC = 1.5 * 2**30
C1_VERT = H_1P + 64.0
SRC1_K = KAPPA_1P - 128.0
ACT_COLS = int(os.environ.get("HEPT_ACT_COLS", "1536"))  # ACT exp col span
DVE_COLS = 2048 - ACT_COLS         # DVE one-pass exp col span (lane-3 side)

_NC_CACHE = {}


# columns per super-tile in the combined input tensor (fp16):
#   q-stacks SUPER_Q*128 | k-stacks SUPER_Q*128 | v SUPER_Q*4*24
SUP_Q_COLS = SUPER_Q * 128
SUP_V_COLS = SUPER_Q * 4 * 24
SUP_COLS = 2 * SUP_Q_COLS + SUP_V_COLS

# per-GROUP input layout (16 blocks): q 4*128 | k 4*128 | v 16*24
GRP_Q = 512
GRP_V = 384
GRP_COLS = 2 * GRP_Q + GRP_V       # 1408
NGROUPS = NQUAD // 4               # 48 groups per core


def _register_exp_ops():
    """Register the two custom-DVE exp ops into concourse.dve_ops.

    pass1 (EXP2K_HEPT): out_i32 = (maxx(P + M, M-126) - (M-127)) * 2^23
      = (clamp(round(P), -126..) + 127) * 2^23 — exactly the fp32 bit
      pattern of 2^K, materialised by the f32->int32 store convert.
    pass2 (EXP2P_HEPT): out_f16 = A*((frac + c)^2 + c^2) * Src1 where
      frac = P - round(P) recomputed via the same magic, Src1 = the f32
      view of pass1's output. Minimax max rel err 2.55e-3.
    """
    from concourse import dve_ops
    from concourse.dve_spec import (
        Spec, Src0, Src1, C0, C1, C2, One, lower, maxx, sq,
        _has_src1,
    )
    from concourse.dve_uop import DveOpSpec

    if "EXP2K_HEPT" in dve_ops._SUB_OPCODE_FOR_NAME:
        by_name = {op.name: op for op in dve_ops.OPS}
        return by_name["EXP2K_HEPT"], by_name["EXP2P_HEPT"]

    def mk(name, spec):
        row = dve_ops._CUSTOM_DVE_ROW_BASE + len(dve_ops.OPS)
        dve_ops._SUB_OPCODE_FOR_NAME[name] = row
        shas = {}
        for ver in ("v3", "v4"):
            try:
                uops = lower(spec, ver=ver)
                shas[ver] = DveOpSpec(
                    name=name, opcode=row, uops=uops,
                    rd1_en=_has_src1(spec)).sha(ver)
            except Exception:
                pass
        op = dve_ops.DveOp(name, spec, subdim=False, uops_sha=shas)
        dve_ops.OPS.append(op)
        dve_ops.CUSTOM_DVE_SPECS[name] = spec
        return op

    body1 = (maxx(Src0 + C0, C1) - (C1 - One)) * C2

    def ref1(in0, in1, s0, s1, imm2):
        zc = (in0.astype(np.float32) + np.float32(s0)).astype(np.float32)
        zcc = np.maximum(zc, np.float32(s1))
        return ((zcc - (np.float32(s1) - np.float32(1.0))) *
                np.float32(imm2)).astype(np.float32)

    op1 = mk("EXP2K_HEPT", Spec(body=body1, reference=ref1))

    zc = Src0 + C0
    fc = (Src0 + (C0 - zc)) + C1
    body2 = ((sq(fc) + C1 * C1) * C2) * Src1

    def ref2(in0, in1, s0, s1, imm2):
        z = (in0.astype(np.float32) + np.float32(s0)).astype(np.float32)
        f0 = (in0.astype(np.float32) +
              (np.float32(s0) - z)).astype(np.float32)
        fc_ = (f0 + np.float32(s1)).astype(np.float32)
        return (((fc_ * fc_ + np.float32(s1) * np.float32(s1)) *
                 np.float32(imm2)) * in1).astype(np.float32)

    op2 = mk("EXP2P_HEPT", Spec(body=body2, reference=ref2))
    return op1, op2


def _register_expbits():
    """One-pass exp: out_u16 = bits_bf16(2^P2) for Src0 = G = 128*P2+16320.

    w = round_128(Src0) = 128*(K+128); x = Src0-w = 128*frac-64;
    out = beta*(x+C1)^2 + w + (kappa-128), stored with a saturating
    f32->uint16 convert (deep-underflow negatives become 0)."""
    from concourse import dve_ops
    from concourse.dve_spec import (
        Spec, Src0, Src1, C0, C1, C2, lower, sq, _has_src1,
    )
    from concourse.dve_uop import DveOpSpec

    if "EXPBITS_HEPT" in dve_ops._SUB_OPCODE_FOR_NAME:
        return {op.name: op for op in dve_ops.OPS}["EXPBITS_HEPT"]

    w = (Src0 + C0) - C0
    x = Src0 - w
    u = x + C1
    body = ((sq(u) * C2) + w) + Src1

    def ref(in0, in1, s0, s1, imm2):
        f32 = np.float32
        z = (in0.astype(f32) + f32(s0)).astype(f32)
        wv = (z - f32(s0)).astype(f32)
        xv = (in0.astype(f32) - wv).astype(f32)
        uv = (xv + f32(s1)).astype(f32)
        return (((uv * uv).astype(f32) * f32(imm2) + wv)
                + in1.astype(f32)).astype(f32)

    spec = Spec(body=body, reference=ref)
    row = dve_ops._CUSTOM_DVE_ROW_BASE + len(dve_ops.OPS)
    dve_ops._SUB_OPCODE_FOR_NAME["EXPBITS_HEPT"] = row
    shas = {}
    for ver in ("v3", "v4"):
        try:
            uops = lower(spec, ver=ver)
            shas[ver] = DveOpSpec(name="EXPBITS_HEPT", opcode=row, uops=uops,
                                  rd1_en=_has_src1(spec)).sha(ver)
        except Exception:
            pass
    op = dve_ops.DveOp("EXPBITS_HEPT", spec, subdim=False, uops_sha=shas)
    dve_ops.OPS.append(op)
    dve_ops.CUSTOM_DVE_SPECS["EXPBITS_HEPT"] = spec
    return op


def build_nc_split1p(n_quads=NQUAD, ipool_bufs=8, apool_bufs=3, opool_bufs=4):
    """Per-group column split: ACT exps 12 blocks (cols 0:1536, bf16 out),
    the one-pass DVE bits-exp handles 4 blocks (cols 1536:2048, uint16 out
    bitcast to bf16); mm2 all-bf16; DVE-half mm2+cast emitted first so the
    PSUM recycle chain ends on the smaller ACT-half cast; output DMA on the
    (otherwise idle) gpsimd queue.  Output block order within a group is
    lane-major (l*4+gq) -- host unpermutes."""
    import concourse.mybir as mybir
    import concourse.tile as tile
    from concourse.bacc import Bacc

    op = _register_expbits()
    f16 = mybir.dt.float16
    bf16 = mybir.dt.bfloat16
    f32 = mybir.dt.float32
    u16 = mybir.dt.uint16
    nblk = n_quads * 4
    ngroups = nblk // 16

    nc = Bacc()
    im = nc.declare_dram_parameter("in", [128, ngroups * GRP_COLS], f16,
                                   isOutput=False)
    om = nc.declare_dram_parameter("out", [128, nblk * 24], f16, isOutput=True)

    with tile.TileContext(nc) as tc, ExitStack() as ctx:
        ipool = ctx.enter_context(tc.tile_pool(name="ipool", bufs=ipool_bufs))
        apool = ctx.enter_context(tc.tile_pool(name="apool", bufs=apool_bufs))
        abpool = ctx.enter_context(tc.tile_pool(name="abpool", bufs=apool_bufs))
        opool = ctx.enter_context(tc.tile_pool(name="opool", bufs=opool_bufs))
        dpool = ctx.enter_context(tc.tile_pool(name="dummy", bufs=1))
        # Tile serialises ALL accessors of one PSUM tile, so the ACT half
        # (lanes 0-2) and DVE half (lane 3) live in separate pools; mm2
        # outputs + cast recycle through ps_d so ps_a frees right after its
        # ACTIVATE read (gram(t+2) never waits on mm2/cast of group t).
        pa_pool = ctx.enter_context(tc.tile_pool(name="psA", bufs=2, space="PSUM"))
        pd_pool = ctx.enter_context(tc.tile_pool(name="psD", bufs=2, space="PSUM"))

        # constants + ACT exp-table warm while the first input DMAs fly
        bias_t = dpool.tile([128, 1], f32)
        nc.vector.memset(bias_t, -LN2 * 127.5)
        kap = dpool.tile([128, DVE_COLS], f32)
        nc.vector.memset(kap, SRC1_K)
        dmy = dpool.tile([128, 8], f32)
        nc.vector.memset(dmy, 0.0)
        dmy2 = dpool.tile([128, 8], bf16)
        nc.scalar.activation(out=dmy2, in_=dmy,
                             func=mybir.ActivationFunctionType.Exp,
                             scale=LN2 / 128.0, bias=bias_t[:, 0:1])

        PREFETCH = ipool_bufs - 2
        tiles = {}

        def fetch(g):
            if g >= ngroups or g in tiles:
                return
            in_t = ipool.tile([128, GRP_COLS], f16)
            nc.sync.dma_start(out=in_t, in_=im[:, g * GRP_COLS:(g + 1) * GRP_COLS])
            tiles[g] = in_t

        def consume(st):
            """Device out position p = col//128 (lane-major); natural block
            id there is gq*4+l.  All mm2 outputs land in ps_d[0:384]."""
            ps_a, ps_d, a_t, ab_t, in_t, g = st
            v_t = in_t[:, 2 * GRP_Q:GRP_COLS].bitcast(bf16)
            o_t = opool.tile([128, 16 * 24], f16)

            def mm2(p):
                col = p * 128
                l, gq = col // 512, (col % 512) // 128
                if col >= ACT_COLS:
                    lhs = ab_t[:, col - ACT_COLS:col - ACT_COLS + 128].bitcast(bf16)
                else:
                    lhs = a_t[:, col:col + 128]
                nc.tensor.matmul(
                    out=ps_d[:, p * 24:(p + 1) * 24], lhsT=lhs,
                    rhs=v_t[:, (gq * 4 + l) * 24:(gq * 4 + l + 1) * 24],
                    start=True, stop=True,
                )

            for p in range(12, 16):        # DVE-half first (only needs ab_t)
                mm2(p)
            for p in range(12):
                mm2(p)
            nc.vector.tensor_copy(out=o_t, in_=ps_d[:, 0:384])
            nc.gpsimd.dma_start(out=om[:, g * 384:(g + 1) * 384], in_=o_t)

        fetch(0)
        fetch(1)
        fetched = [2]

        def topup(g):
            n = 0
            while (fetched[0] < ngroups and fetched[0] <= g + PREFETCH
                   and n < 2):
                fetch(fetched[0])
                fetched[0] += 1
                n += 1

        pend = None
        for g in range(ngroups):
            in_t = tiles.pop(g)
            q_t = in_t[:, 0:GRP_Q]
            k_t = in_t[:, GRP_Q:2 * GRP_Q]
            ps_a = pa_pool.tile([128, ACT_COLS], f32)
            ps_d = pd_pool.tile([128, DVE_COLS], f32)
            for gq in range(4):
                for l in range(4):
                    col = l * 512 + gq * 128
                    out_ap = (ps_a[:, col:col + 128] if col < ACT_COLS
                              else ps_d[:, col - ACT_COLS:col - ACT_COLS + 128])
                    nc.tensor.matmul(
                        out=out_ap,
                        lhsT=k_t[32 * l:32 * l + 32, gq * 128:(gq + 1) * 128],
                        rhs=q_t[32 * l:32 * l + 32, gq * 128:(gq + 1) * 128],
                        start=True, stop=True,
                        tile_position=(32 * l, 0),
                    )
            topup(g)
            a_t = apool.tile([128, ACT_COLS], bf16)
            nc.scalar.activation(
                out=a_t, in_=ps_a,
                func=mybir.ActivationFunctionType.Exp,
                scale=LN2 / 128.0, bias=bias_t[:, 0:1],
            )
            ab_t = abpool.tile([128, DVE_COLS], u16)
            nc.vector._custom_dve(
                op, out=ab_t, in0=ps_d, in1=kap,
                s0=C0_MAGIC, s1=C1_VERT, imm2=BETA_1P,
            )
            if pend is not None:
                consume(pend)
            pend = (ps_a, ps_d, a_t, ab_t, in_t, g)
        consume(pend)
    nc.finalize()
    return nc


def build_nc(n_quads=NQUAD, ipool_bufs=3, apool_bufs=3, opool_bufs=3):
    """Build the per-core Bass module (same NEFF for all 8 cores).

    Constraints baked in:
    - One combined input DMA per super-tile (q|k|v).
    - Row-tiled (tile_position) matmuls sharing a PSUM *bank* crash the
      device -> the 4 lanes of a quad write 4 distinct banks: lanes 0-2
      into ps_act (3 banks), lane 3 into ps_dve (1 bank).
    - Every accessor of one PSUM tile is serialised by the framework,
      so ACT work and DVE work live on separate PSUM tiles.
    """
    import concourse.mybir as mybir
    import concourse.tile as tile
    from concourse.bacc import Bacc

    op1, op2 = _register_exp_ops()

    f16 = mybir.dt.float16
    f32 = mybir.dt.float32
    i32 = mybir.dt.int32
    nblk = n_quads * 4
    assert n_quads % SUPER_Q == 0
    n_super = n_quads // SUPER_Q

    nc = Bacc()
    im = nc.declare_dram_parameter("in", [128, n_super * SUP_COLS], f16,
                                   isOutput=False)
    om = nc.declare_dram_parameter("out", [128, nblk * 24], f16, isOutput=True)

    with tile.TileContext(nc) as tc, ExitStack() as ctx:
        ipool = ctx.enter_context(tc.tile_pool(name="ipool", bufs=ipool_bufs))
        apool = ctx.enter_context(tc.tile_pool(name="apool", bufs=apool_bufs))
        adpool = ctx.enter_context(tc.tile_pool(name="adpool", bufs=apool_bufs))
        ypool = ctx.enter_context(tc.tile_pool(name="ypool", bufs=apool_bufs))
        opool = ctx.enter_context(tc.tile_pool(name="opool", bufs=opool_bufs))
        pa_pool = ctx.enter_context(tc.tile_pool(name="psA", bufs=2, space="PSUM"))
        pd_pool = ctx.enter_context(tc.tile_pool(name="psD", bufs=2, space="PSUM"))

        def emit_mm2_and_drain(st):
            """Consume stage st = (ps_a, ps_d, aa, ad, v_t, og_base, b0):
            mm2 for all 16 blocks, then cast + DMA the outputs."""
            ps_a, ps_d, aa, ad, v_t, og4, b0 = st
            for bb in range(16):
                if bb < NACT_BLK:
                    gq, l = bb // 3, bb % 3
                    lhs = aa[:, (l * 512 + gq * 128):(l * 512 + gq * 128 + 128)]
                    out_ap = ps_a[:, bb * 24:bb * 24 + 24]
                else:
                    gq, l = bb - NACT_BLK, 3
                    lhs = ad[:, gq * 128:gq * 128 + 128]
                    out_ap = ps_d[:, (bb - NACT_BLK) * 24:
                                  (bb - NACT_BLK) * 24 + 24]
                b_local = (og4 + gq) * 4 + l
                nc.tensor.matmul(
                    out=out_ap,
                    lhsT=lhs,
                    rhs=v_t[:, b_local * 24:(b_local + 1) * 24],
                    start=True, stop=True,
                )
            o_t = opool.tile([128, 16 * 24], f16)
            if CAST_ACT > 0:
                nc.scalar.copy(
                    out=o_t[:, 0:CAST_ACT], in_=ps_a[:, 0:CAST_ACT])
                nc.vector.tensor_copy(
                    out=o_t[:, CAST_ACT:288], in_=ps_a[:, CAST_ACT:288])
            else:
                nc.vector.tensor_copy(out=o_t[:, 0:288], in_=ps_a[:, 0:288])
            nc.vector.tensor_copy(out=o_t[:, 288:384], in_=ps_d[:, 0:96])
            nc.sync.dma_start(out=om[:, b0 * 24:(b0 + 16) * 24], in_=o_t)

        pend = None                             # previous group's stage
        for s in range(n_super):
            in_t = ipool.tile([128, SUP_COLS], f16)
            c0 = s * SUP_COLS
            nc.sync.dma_start(out=in_t, in_=im[:, c0:c0 + SUP_COLS])
            q_t = in_t[:, 0:SUP_Q_COLS]
            k_t = in_t[:, SUP_Q_COLS:2 * SUP_Q_COLS]
            v_t = in_t[:, 2 * SUP_Q_COLS:SUP_COLS]

            for og in range(4):                 # 4 groups of 4 quads (16 blk)
                ps_a = pa_pool.tile([128, 1536], f32)   # banks for lanes 0-2
                ps_d = pd_pool.tile([128, 512], f32)    # bank for lane 3
                for gq in range(4):             # ACT lanes first
                    qd = og * 4 + gq
                    for l in range(3):
                        col = l * 512 + gq * 128
                        nc.tensor.matmul(
                            out=ps_a[:, col:col + 128],
                            lhsT=k_t[32 * l:32 * l + 32, qd * 128:(qd + 1) * 128],
                            rhs=q_t[32 * l:32 * l + 32, qd * 128:(qd + 1) * 128],
                            start=True, stop=True,
                            tile_position=(32 * l, 0),
                        )
                for gq in range(4):             # DVE lane last
                    qd = og * 4 + gq
                    nc.tensor.matmul(
                        out=ps_d[:, gq * 128:gq * 128 + 128],
                        lhsT=k_t[96:128, qd * 128:(qd + 1) * 128],
                        rhs=q_t[96:128, qd * 128:(qd + 1) * 128],
                        start=True, stop=True,
                        tile_position=(96, 0),
                    )
                aa = apool.tile([128, 1536], f16)
                ad = adpool.tile([128, 512], f16)
                y_t = ypool.tile([128, 512], f32)
                nc.scalar.activation(
                    out=aa, in_=ps_a,
                    func=mybir.ActivationFunctionType.Exp,
                    scale=LN2,
                )
                nc.vector._custom_dve(
                    op1, out=y_t.bitcast(i32), in0=ps_d,
                    s0=DVE_M, s1=DVE_CLO, imm2=DVE_E23,
                )
                nc.vector._custom_dve(
                    op2, out=ad, in0=ps_d,
                    in1=y_t, s0=DVE_M, s1=DVE_C, imm2=DVE_A,
                )
                if pend is not None:            # consume the PREVIOUS group
                    emit_mm2_and_drain(pend)
                pend = (ps_a, ps_d, aa, ad, v_t, og * 4, (s * 4 + og) * 16)
        emit_mm2_and_drain(pend)
    nc.finalize()
    return nc


def build_nc_allact(n_quads=NQUAD, ipool_bufs=8, apool_bufs=3, opool_bufs=3):
    """Single-PSUM-pool variant: ALL exp on ScalarE (one 2048-col ACT
    instr per group keeps the scalar queue saturated), casts on VectorE,
    mm2 + drain software-pipelined one group behind the fills so every
    wait is pre-posted when its consumer reaches the queue head.
    Input arrives in per-group chunks (1408 cols) DMA'd several groups
    ahead so super-boundary stalls disappear."""
    import concourse.mybir as mybir
    import concourse.tile as tile
    from concourse.bacc import Bacc

    f16 = mybir.dt.float16
    f32 = mybir.dt.float32
    nblk = n_quads * 4
    ngroups = nblk // 16

    nc = Bacc()
    im = nc.declare_dram_parameter("in", [128, ngroups * GRP_COLS], f16,
                                   isOutput=False)
    om = nc.declare_dram_parameter("out", [128, nblk * 24], f16, isOutput=True)

    with tile.TileContext(nc) as tc, ExitStack() as ctx:
        ipool = ctx.enter_context(tc.tile_pool(name="ipool", bufs=ipool_bufs))
        apool = ctx.enter_context(tc.tile_pool(name="apool", bufs=apool_bufs))
        opool = ctx.enter_context(tc.tile_pool(name="opool", bufs=opool_bufs))
        dpool = ctx.enter_context(tc.tile_pool(name="dummy", bufs=1))
        ps1pool = ctx.enter_context(tc.tile_pool(name="ps1", bufs=2, space="PSUM"))

        # warm the exp ACT table while the first input DMAs are in flight
        dmy = dpool.tile([128, 8], f32)
        nc.vector.memset(dmy, 0.0)
        dmy2 = dpool.tile([128, 8], f16)
        nc.scalar.activation(out=dmy2, in_=dmy,
                             func=mybir.ActivationFunctionType.Exp)

        PREFETCH = ipool_bufs - 2
        tiles = {}

        def fetch(g):
            if g >= ngroups or g in tiles:
                return
            in_t = ipool.tile([128, GRP_COLS], f16)
            nc.sync.dma_start(out=in_t, in_=im[:, g * GRP_COLS:(g + 1) * GRP_COLS])
            tiles[g] = in_t

        def consume(st):
            ps1, a_t, in_t, b0 = st
            v_t = in_t[:, 2 * GRP_Q:GRP_COLS]
            for bb in range(16):
                gq, l = bb // 4, bb % 4
                acol = l * 512 + gq * 128
                nc.tensor.matmul(
                    out=ps1[:, bb * 24:(bb + 1) * 24],
                    lhsT=a_t[:, acol:acol + 128],
                    rhs=v_t[:, (gq * 4 + l) * 24:(gq * 4 + l + 1) * 24],
                    start=True, stop=True,
                )
            o_t = opool.tile([128, 16 * 24], f16)
            nc.vector.tensor_copy(out=o_t, in_=ps1[:, 0:384])
            nc.sync.dma_start(out=om[:, b0 * 24:(b0 + 16) * 24], in_=o_t)

        # Stagger the prefetch buildup: only groups 0-1 before the loop so
        # group 0's chunk isn't queued behind 2MB of lookahead traffic;
        # depth then grows by up to 2 fetches per group until PREFETCH.
        fetch(0)
        fetch(1)
        fetched = [2]

        def topup(g):
            n = 0
            while (fetched[0] < ngroups and fetched[0] <= g + PREFETCH
                   and n < 2):
                fetch(fetched[0])
                fetched[0] += 1
                n += 1

        pend = None
        for g in range(ngroups):
            in_t = tiles.pop(g)
            q_t = in_t[:, 0:GRP_Q]
            k_t = in_t[:, GRP_Q:2 * GRP_Q]
            ps1 = ps1pool.tile([128, 2048], f32)
            for gq in range(4):
                for l in range(4):
                    col = l * 512 + gq * 128
                    nc.tensor.matmul(
                        out=ps1[:, col:col + 128],
                        lhsT=k_t[32 * l:32 * l + 32, gq * 128:(gq + 1) * 128],
                        rhs=q_t[32 * l:32 * l + 32, gq * 128:(gq + 1) * 128],
                        start=True, stop=True,
                        tile_position=(32 * l, 0),
                    )
            topup(g)
            a_t = apool.tile([128, 2048], f16)
            nc.scalar.activation(
                out=a_t, in_=ps1,
                func=mybir.ActivationFunctionType.Exp,
                scale=LN2,
            )
            if pend is not None:
                consume(pend)
            pend = (ps1, a_t, in_t, g * 16)
        consume(pend)
    nc.finalize()
    return nc


def build_nc_split2(n_quads=NQUAD, ipool_bufs=8, apool_bufs=3, opool_bufs=3):
    """Per-group-DMA + software-pipelined variant with the exp split
    across ScalarE (lanes 0-2, 3 PSUM banks) and VectorE custom-DVE
    (lane 3, 1 PSUM bank).  Output casts: ps_a's 288 cols on ACT (its
    own pool), ps_d's 96 cols on DVE."""
    import concourse.mybir as mybir
    import concourse.tile as tile
    from concourse.bacc import Bacc

    op1, op2 = _register_exp_ops()
    f16 = mybir.dt.float16
    f32 = mybir.dt.float32
    i32 = mybir.dt.int32
    nblk = n_quads * 4
    ngroups = nblk // 16

    nc = Bacc()
    im = nc.declare_dram_parameter("in", [128, ngroups * GRP_COLS], f16,
                                   isOutput=False)
    om = nc.declare_dram_parameter("out", [128, nblk * 24], f16, isOutput=True)

    with tile.TileContext(nc) as tc, ExitStack() as ctx:
        ipool = ctx.enter_context(tc.tile_pool(name="ipool", bufs=ipool_bufs))
        apool = ctx.enter_context(tc.tile_pool(name="apool", bufs=apool_bufs))
        adpool = ctx.enter_context(tc.tile_pool(name="adpool", bufs=apool_bufs))
        ypool = ctx.enter_context(tc.tile_pool(name="ypool", bufs=apool_bufs))
        opool = ctx.enter_context(tc.tile_pool(name="opool", bufs=opool_bufs))
        dpool = ctx.enter_context(tc.tile_pool(name="dummy", bufs=1))
        pa_pool = ctx.enter_context(tc.tile_pool(name="psA", bufs=2, space="PSUM"))
        pd_pool = ctx.enter_context(tc.tile_pool(name="psD", bufs=2, space="PSUM"))

        # warm the exp ACT table while the first input DMA is in flight
        dmy = dpool.tile([128, 8], f32)
        nc.vector.memset(dmy, 0.0)
        dmy2 = dpool.tile([128, 8], f16)
        nc.scalar.activation(out=dmy2, in_=dmy,
                             func=mybir.ActivationFunctionType.Exp)

        PREFETCH = ipool_bufs - 2
        tiles = {}

        def fetch(g):
            if g >= ngroups or g in tiles:
                return
            in_t = ipool.tile([128, GRP_COLS], f16)
            nc.sync.dma_start(out=in_t, in_=im[:, g * GRP_COLS:(g + 1) * GRP_COLS])
            tiles[g] = in_t

        def consume(st):
            ps_a, ps_d, aa, ad, in_t, b0 = st
            v_t = in_t[:, 2 * GRP_Q:GRP_COLS]
            for bb in range(16):
                if bb < NACT_BLK:
                    gq, l = bb // 3, bb % 3
                    lhs = aa[:, (l * 512 + gq * 128):(l * 512 + gq * 128 + 128)]
                    out_ap = ps_a[:, bb * 24:bb * 24 + 24]
                else:
                    gq, l = bb - NACT_BLK, 3
                    lhs = ad[:, gq * 128:gq * 128 + 128]
                    out_ap = ps_d[:, (bb - NACT_BLK) * 24:
                                  (bb - NACT_BLK) * 24 + 24]
                nc.tensor.matmul(
                    out=out_ap,
                    lhsT=lhs,
                    rhs=v_t[:, (gq * 4 + l) * 24:(gq * 4 + l + 1) * 24],
                    start=True, stop=True,
                )
            o_t = opool.tile([128, 16 * 24], f16)
            nc.scalar.copy(out=o_t[:, 0:96], in_=ps_a[:, 0:96])
            nc.vector.tensor_copy(out=o_t[:, 96:288], in_=ps_a[:, 96:288])
            nc.vector.tensor_copy(out=o_t[:, 288:384], in_=ps_d[:, 0:96])
            nc.sync.dma_start(out=om[:, b0 * 24:(b0 + 16) * 24], in_=o_t)

        for g in range(PREFETCH):
            fetch(g)
        pend = None
        for g in range(ngroups):
            in_t = tiles.pop(g)
            q_t = in_t[:, 0:GRP_Q]
            k_t = in_t[:, GRP_Q:2 * GRP_Q]
            ps_a = pa_pool.tile([128, 1536], f32)
            ps_d = pd_pool.tile([128, 512], f32)
            for gq in range(4):
                for l in range(3):
                    col = l * 512 + gq * 128
                    nc.tensor.matmul(
                        out=ps_a[:, col:col + 128],
                        lhsT=k_t[32 * l:32 * l + 32, gq * 128:(gq + 1) * 128],
                        rhs=q_t[32 * l:32 * l + 32, gq * 128:(gq + 1) * 128],
                        start=True, stop=True,
                        tile_position=(32 * l, 0),
                    )
            for gq in range(4):
                nc.tensor.matmul(
                    out=ps_d[:, gq * 128:gq * 128 + 128],
                    lhsT=k_t[96:128, gq * 128:(gq + 1) * 128],
                    rhs=q_t[96:128, gq * 128:(gq + 1) * 128],
                    start=True, stop=True,
                    tile_position=(96, 0),
                )
            fetch(g + PREFETCH)
            aa = apool.tile([128, 1536], f16)
            ad = adpool.tile([128, 512], f16)
            y_t = ypool.tile([128, 512], f32)
            nc.scalar.activation(
                out=aa, in_=ps_a,
                func=mybir.ActivationFunctionType.Exp,
                scale=LN2,
            )
            nc.vector._custom_dve(
                op1, out=y_t.bitcast(i32), in0=ps_d,
                s0=DVE_M, s1=DVE_CLO, imm2=DVE_E23,
            )
            nc.vector._custom_dve(
                op2, out=ad, in0=ps_d,
                in1=y_t, s0=DVE_M, s1=DVE_C, imm2=DVE_A,
            )
            if pend is not None:
                consume(pend)
            pend = (ps_a, ps_d, aa, ad, in_t, g * 16)
        consume(pend)
    nc.finalize()
    return nc


def _get_nc(n_quads=NQUAD):
    key = (n_quads, MODE)
    if key not in _NC_CACHE:
        if MODE == "split1p":
            _NC_CACHE[key] = build_nc_split1p(n_quads)
        elif MODE == "allact":
            _NC_CACHE[key] = build_nc_allact(n_quads)
        elif MODE == "split2":
            _NC_CACHE[key] = build_nc_split2(n_quads)
        else:
            _NC_CACHE[key] = build_nc(n_quads)
    return _NC_CACHE[key]


# ---------------- host-side preparation ----------------

def _sort_indices(query, key, combined_shifts, alpha):
    """Replicate the reference's hash + argsort with jax on CPU.

    Uses the exact same jnp ops the reference uses so the fp32 values
    (and therefore the argsort permutations) match bit-for-bit.
    """
    import jax
    import jax.numpy as jnp

    cpu = jax.devices("cpu")[0]
    with jax.default_device(cpu):
        q = jnp.asarray(query)
        k = jnp.asarray(key)
        al = jnp.asarray(alpha)
        cs_i = jnp.asarray(combined_shifts)
        q_hashed = jnp.einsum('hnd,hdr->rhn', q, al)
        k_hashed = jnp.einsum('hnd,hdr->rhn', k, al)
        max_shift = jnp.maximum(q_hashed.max(-1, keepdims=True),
                                k_hashed.max(-1, keepdims=True))
        min_shift = jnp.minimum(q_hashed.min(-1, keepdims=True),
                                k_hashed.min(-1, keepdims=True))
        hash_shift = max_shift - min_shift
        cs = cs_i.astype(q_hashed.dtype) * hash_shift
        q_pos = np.asarray(jnp.argsort(q_hashed + cs, axis=-1))
        k_pos = np.asarray(jnp.argsort(k_hashed + cs, axis=-1))
    return q_pos, k_pos


def _split16(x):
    hi = x.astype(np.float16)
    lo = (x - hi.astype(np.float32)).astype(np.float16)
    return hi, lo


def _build_stack(s_qk, is_k, scale=SQRT_LOG2E, offset=0.0):
    """(UNITS*NB, 128, 27) f32 -> (UNITS*NB, 32, 128) fp16 stack.

    Data rows are pre-scaled by `scale` so the on-device Gram is
    scale^2 * (-0.5*||q-k||^2); the norm rows use the scaled fp16 data
    and carry `offset` (split1p bakes +8160 per side into the Gram).
    """
    nblk = s_qk.shape[0]
    hi = (s_qk * np.float32(scale)).astype(np.float16)  # (b, i, d)
    sqm = -0.5 * np.einsum('bid,bid->bi', hi.astype(np.float32),
                           hi.astype(np.float32)) + np.float32(offset)
    sq_hi, sq_lo = _split16(sqm)
    st = np.zeros((nblk, KROWS, BLOCK), np.float16)
    st[:, :D_QK, :] = hi.transpose(0, 2, 1)            # rows 0-26: x^T
    if is_k:
        st[:, 27, :] = 1.0                             # pair of q's sq rows
        st[:, 28, :] = 1.0
        st[:, 29, :] = sq_hi                           # -0.5*||k||^2 hi
        st[:, 30, :] = sq_lo
    else:
        st[:, 27, :] = sq_hi                           # -0.5*||q||^2 hi
        st[:, 28, :] = sq_lo
        st[:, 29, :] = 1.0                             # pair of k's sq rows
        st[:, 30, :] = 1.0
    return st


def _pack_core(stack_blocks):
    """(768, 32, 128) -> (128, 192*128): partition = lane*32+row,
    free = quad*128 + col."""
    return (stack_blocks.reshape(NQUAD, 4, KROWS, BLOCK)
            .transpose(1, 2, 0, 3)
            .reshape(128, NQUAD * BLOCK))


# om block order within each 16-block group: 12 ACT blocks (lanes 0-2 in
# (quad, lane) order) then 4 DVE blocks (lane 3).  _OM_PERM[bb] = b_local
# offset within the group's 16 blocks.
_OM_PERM = np.array(
    [q * 4 + l for q in range(4) for l in range(3)] +
    [q * 4 + 3 for q in range(4)], dtype=np.int64)


def prepare_in_maps(query, key, value, combined_shifts, alpha):
    query = np.ascontiguousarray(np.asarray(query), dtype=np.float32)
    key = np.ascontiguousarray(np.asarray(key), dtype=np.float32)
    value = np.ascontiguousarray(np.asarray(value), dtype=np.float32)
    combined_shifts = np.asarray(combined_shifts)
    alpha = np.asarray(alpha, dtype=np.float32)

    q_pos, k_pos = _sort_indices(query, key, combined_shifts, alpha)

    h_idx = np.arange(N_HEADS)[None, :, None]
    s_query = query[h_idx, q_pos].reshape(UNITS * NB, BLOCK, D_QK)
    s_key = key[h_idx, k_pos].reshape(UNITS * NB, BLOCK, D_QK)
    s_value = value[h_idx, k_pos].reshape(UNITS * NB, BLOCK, DIM_PER_HEAD)

    if MODE == "split1p":
        qstack = _build_stack(s_query, is_k=False, scale=SC2, offset=G_OFS / 2)
        kstack = _build_stack(s_key, is_k=True, scale=SC2, offset=G_OFS / 2)
        # v as bf16 bit patterns carried in the f16-typed input buffer
        vb = np.ascontiguousarray(s_value, np.float32).view(np.uint32)
        vb = ((vb + 0x7FFF + ((vb >> 16) & 1)) >> 16).astype(np.uint16)
        v16 = vb.view(np.float16)
    else:
        qstack = _build_stack(s_query, is_k=False)
        kstack = _build_stack(s_key, is_k=True)
        v16 = s_value.astype(np.float16)

    in_maps = []
    for c in range(N_CORES):
        b0, b1 = c * NBLK, (c + 1) * NBLK
        qp = _pack_core(qstack[b0:b1])              # [128, NQUAD*128]
        kp = _pack_core(kstack[b0:b1])
        vp = v16[b0:b1].transpose(1, 0, 2).reshape(128, NBLK * 24)
        combined = np.empty((128, NGROUPS * GRP_COLS), np.float16)
        for g in range(NGROUPS):
            c0 = g * GRP_COLS
            combined[:, c0:c0 + GRP_Q] = qp[:, g * GRP_Q:(g + 1) * GRP_Q]
            combined[:, c0 + GRP_Q:c0 + 2 * GRP_Q] = \
                kp[:, g * GRP_Q:(g + 1) * GRP_Q]
            combined[:, c0 + 2 * GRP_Q:c0 + GRP_COLS] = \
                vp[:, g * GRP_V:(g + 1) * GRP_V]
        in_maps.append({"in": combined})
    return in_maps


def assemble_output(results):
    """results: list of 8 dicts with 'out' [128, 768*24] f16 in the
    permuted (ACT-blocks-first) group order."""
    ngroups = NBLK // 16
    if MODE == "allact":
        om_perm = np.arange(16)
    elif MODE == "split1p":
        # device col p = l*4+gq holds natural block gq*4+l
        om_perm = np.array([(p % 4) * 4 + p // 4 for p in range(16)],
                          dtype=np.int64)
    else:
        om_perm = _OM_PERM
    perm = (np.arange(ngroups)[:, None] * 16 + om_perm[None, :]).ravel()
    inv = np.empty_like(perm)
    inv[perm] = np.arange(NBLK)
    out = np.empty((UNITS, NB, BLOCK, DIM_PER_HEAD), np.float32)
    for c in range(N_CORES):
        so = np.asarray(results[c]["out"]).astype(np.float32)
        so = so.reshape(128, NBLK, 24)[:, inv, :]
        out[c * UPC:(c + 1) * UPC] = (
            so.transpose(1, 0, 2).reshape(UPC, NB, BLOCK, DIM_PER_HEAD))
    return out.reshape(N_HASHES, N_HEADS, NB, BLOCK, DIM_PER_HEAD)


def run(query, key, value, combined_shifts, alpha, trace=False):
    from concourse.bass_utils import run_bass_kernel_spmd

    in_maps = prepare_in_maps(query, key, value, combined_shifts, alpha)
    nc = _get_nc()
    res = run_bass_kernel_spmd(
        nc, in_maps, core_ids=list(range(N_CORES)), trace=trace)
    out = assemble_output(res.results)
    return out, res


def kernel(query, key, value, combined_shifts, alpha):
    out, _ = run(query, key, value, combined_shifts, alpha,
                 trace=bool(int(os.environ.get("HEPT_TRACE", "0"))))
    return out



# revision 15
# speedup vs baseline: 1.4108x; 1.0213x over previous
"""HEPT sparse attention for Trainium2 — 8-core SPMD Bass kernel.

Reference computation (per hash-round r, head h):
  hash q/k via shared projection, argsort, gather into blocks of 128,
  blocked RBF attention: so = exp(-0.5*||q_i-k_j||^2) @ v.

Strategy (MODE="allact", the measured-fastest variant):
  - Host: bitwise-exact hash + argsort (jax CPU, identical ops to the
    reference), gather, fp16 quantization (data rows pre-scaled by
    sqrt(log2e) so the device Gram is in log2 units), per-group layout
    packing (q|k|v chunks of 1408 cols per 16-block group).
  - Device (per core, 3 of the 24 (r,h) units = 768 blocks): the whole
    log2e*-0.5*||q-k||^2 Gram of one block is ONE K=32 fp16 matmul
    (rows 0-26 data, rows 27-30 squared-norm terms hi/lo paired with
    ones rows); 4 blocks share the PE via row tiling into 4 PSUM banks.
    exp runs as one 2048-col ScalarE ACT instruction per group
    (exp(ln2*P) = 2^P); mm2 (so = A @ v, A fp16 weights) writes back
    into exp-consumed PSUM; VectorE casts f32->f16 for the output DMA.
  - Pipeline: per-group input DMAs prefetched 6 groups ahead, the exp
    ACT table preloaded during the DMA prefill, and mm2 + cast + output
    DMA emitted one group behind the fills, so every wait is pre-posted
    and ScalarE (the bottleneck engine, ~94 us busy) runs back-to-back.
  - A "split2" variant (custom-DVE 2-pass exp for a quarter of the
    Gram on VectorE) measured consistently slower (~127-132 us vs
    ~123 us) due to cross-engine handoff latency on the extra PSUM
    recycle chains, and is kept for reference.
"""

import math
import os
from contextlib import ExitStack

import numpy as np

# ---- problem constants (hardcoded; kernel.py must be self-contained) ----
N_HASHES = 3
N_HEADS = 8
PADDED_SIZE = 32768
BLOCK = 128
DIM_PER_HEAD = 24
D_QK = 27
NB = PADDED_SIZE // BLOCK          # 256 blocks per (r,h)
N_CORES = 8
UNITS = N_HASHES * N_HEADS         # 24 independent (r,h) units
UPC = UNITS // N_CORES             # 3 units per core
NBLK = UPC * NB                    # 768 blocks per core
NQUAD = NBLK // 4                  # 192 quads per core (4 blocks/quad)
SUPER_Q = 16                       # quads per super-tile (64 blocks)
KROWS = 32                         # stacked contraction rows per block

LOG2E = 1.4426950408889634
SQRT_LOG2E = math.sqrt(LOG2E)
LN2 = 0.6931471805599453

# custom-DVE exp constants
DVE_M = 12582912.0                 # 1.5 * 2^23 round-to-int magic
DVE_CLO = DVE_M - 126.0            # clamp: K >= -126
DVE_E23 = 8388608.0                # 2^23
DVE_A = 0.24973141119916378        # minimax A*((s+c)^2 + c^2) ~ 2^s, |s|<=.5
DVE_C = 1.414837949227267

NACT_BLK = 12                      # blocks/group exp'd on ACT (lanes 0-2)
NDVE_BLK = 4                       # blocks/group exp'd on DVE (lane 3)
CAST_ACT = 0                       # output cols cast on ACT (0 = all on DVE)
MODE = os.environ.get("HEPT_MODE", "split1p")

# ---- split1p mode: one-pass DVE bits-exp + ACT exp column split ----
# Gram G = 128*log2(e)*(-0.5*||q-k||^2) + 16320 (offset baked via norm rows).
# ACT path: exp(G*ln2/128 - 127.5*ln2) = 2^P2, written bf16.
# DVE path: one 8-slice custom op emits the uint16 BIT PATTERN of the bf16
#   value 2^P2 (magic-add exponent extraction + minimax quadratic mantissa),
#   negative (underflow) results clamp to 0 via the saturating uint16 store.
SC2 = math.sqrt(128.0 * LOG2E)     # host pre-scale for q/k data rows
G_OFS = 16320.0                    # = 128*127.5, split 8160 per side
BETA_1P = 0.00268750865            # minimax quad: beta*(y+H)^2 + kappa
H_1P = 121.124593
KAPPA_1P = -39.1119528
C0_MAGIC = 1.5 * 2**30
C1_VERT = H_1P + 64.0
SRC1_K = KAPPA_1P - 128.0
ACT_COLS = int(os.environ.get("HEPT_ACT_COLS", "1536"))  # ACT exp col span
DVE_COLS = 2048 - ACT_COLS         # DVE one-pass exp col span (lane-3 side)

_NC_CACHE = {}


# columns per super-tile in the combined input tensor (fp16):
#   q-stacks SUPER_Q*128 | k-stacks SUPER_Q*128 | v SUPER_Q*4*24
SUP_Q_COLS = SUPER_Q * 128
SUP_V_COLS = SUPER_Q * 4 * 24
SUP_COLS = 2 * SUP_Q_COLS + SUP_V_COLS

# per-GROUP input layout (16 blocks): q 4*128 | k 4*128 | v 16*24
GRP_Q = 512
GRP_V = 384
GRP_COLS = 2 * GRP_Q + GRP_V       # 1408
NGROUPS = NQUAD // 4               # 48 groups per core


def _register_exp_ops():
    """Register the two custom-DVE exp ops into concourse.dve_ops.

    pass1 (EXP2K_HEPT): out_i32 = (maxx(P + M, M-126) - (M-127)) * 2^23
      = (clamp(round(P), -126..) + 127) * 2^23 — exactly the fp32 bit
      pattern of 2^K, materialised by the f32->int32 store convert.
    pass2 (EXP2P_HEPT): out_f16 = A*((frac + c)^2 + c^2) * Src1 where
      frac = P - round(P) recomputed via the same magic, Src1 = the f32
      view of pass1's output. Minimax max rel err 2.55e-3.
    """
    from concourse import dve_ops
    from concourse.dve_spec import (
        Spec, Src0, Src1, C0, C1, C2, One, lower, maxx, sq,
        _has_src1,
    )
    from concourse.dve_uop import DveOpSpec

    if "EXP2K_HEPT" in dve_ops._SUB_OPCODE_FOR_NAME:
        by_name = {op.name: op for op in dve_ops.OPS}
        return by_name["EXP2K_HEPT"], by_name["EXP2P_HEPT"]

    def mk(name, spec):
        row = dve_ops._CUSTOM_DVE_ROW_BASE + len(dve_ops.OPS)
        dve_ops._SUB_OPCODE_FOR_NAME[name] = row
        shas = {}
        for ver in ("v3", "v4"):
            try:
                uops = lower(spec, ver=ver)
                shas[ver] = DveOpSpec(
                    name=name, opcode=row, uops=uops,
                    rd1_en=_has_src1(spec)).sha(ver)
            except Exception:
                pass
        op = dve_ops.DveOp(name, spec, subdim=False, uops_sha=shas)
        dve_ops.OPS.append(op)
        dve_ops.CUSTOM_DVE_SPECS[name] = spec
        return op

    body1 = (maxx(Src0 + C0, C1) - (C1 - One)) * C2

    def ref1(in0, in1, s0, s1, imm2):
        zc = (in0.astype(np.float32) + np.float32(s0)).astype(np.float32)
        zcc = np.maximum(zc, np.float32(s1))
        return ((zcc - (np.float32(s1) - np.float32(1.0))) *
                np.float32(imm2)).astype(np.float32)

    op1 = mk("EXP2K_HEPT", Spec(body=body1, reference=ref1))

    zc = Src0 + C0
    fc = (Src0 + (C0 - zc)) + C1
    body2 = ((sq(fc) + C1 * C1) * C2) * Src1

    def ref2(in0, in1, s0, s1, imm2):
        z = (in0.astype(np.float32) + np.float32(s0)).astype(np.float32)
        f0 = (in0.astype(np.float32) +
              (np.float32(s0) - z)).astype(np.float32)
        fc_ = (f0 + np.float32(s1)).astype(np.float32)
        return (((fc_ * fc_ + np.float32(s1) * np.float32(s1)) *
                 np.float32(imm2)) * in1).astype(np.float32)

    op2 = mk("EXP2P_HEPT", Spec(body=body2, reference=ref2))
    return op1, op2


def _register_expbits():
    """One-pass exp: out_u16 = bits_bf16(2^P2) for Src0 = G = 128*P2+16320.

    w = round_128(Src0) = 128*(K+128); x = Src0-w = 128*frac-64;
    out = beta*(x+C1)^2 + w + (kappa-128), stored with a saturating
    f32->uint16 convert (deep-underflow negatives become 0)."""
    from concourse import dve_ops
    from concourse.dve_spec import (
        Spec, Src0, Src1, C0, C1, C2, lower, sq, _has_src1,
    )
    from concourse.dve_uop import DveOpSpec

    if "EXPBITS_HEPT" in dve_ops._SUB_OPCODE_FOR_NAME:
        return {op.name: op for op in dve_ops.OPS}["EXPBITS_HEPT"]

    w = (Src0 + C0) - C0
    x = Src0 - w
    u = x + C1
    body = ((sq(u) * C2) + w) + Src1

    def ref(in0, in1, s0, s1, imm2):
        f32 = np.float32
        z = (in0.astype(f32) + f32(s0)).astype(f32)
        wv = (z - f32(s0)).astype(f32)
        xv = (in0.astype(f32) - wv).astype(f32)
        uv = (xv + f32(s1)).astype(f32)
        return (((uv * uv).astype(f32) * f32(imm2) + wv)
                + in1.astype(f32)).astype(f32)

    spec = Spec(body=body, reference=ref)
    row = dve_ops._CUSTOM_DVE_ROW_BASE + len(dve_ops.OPS)
    dve_ops._SUB_OPCODE_FOR_NAME["EXPBITS_HEPT"] = row
    shas = {}
    for ver in ("v3", "v4"):
        try:
            uops = lower(spec, ver=ver)
            shas[ver] = DveOpSpec(name="EXPBITS_HEPT", opcode=row, uops=uops,
                                  rd1_en=_has_src1(spec)).sha(ver)
        except Exception:
            pass
    op = dve_ops.DveOp("EXPBITS_HEPT", spec, subdim=False, uops_sha=shas)
    dve_ops.OPS.append(op)
    dve_ops.CUSTOM_DVE_SPECS["EXPBITS_HEPT"] = spec
    return op


def build_nc_split1p(n_quads=NQUAD, ipool_bufs=8, apool_bufs=3, opool_bufs=4):
    """Per-group column split: ACT exps 12 blocks (cols 0:1536, bf16 out),
    the one-pass DVE bits-exp handles 4 blocks (cols 1536:2048, uint16 out
    bitcast to bf16); mm2 all-bf16; DVE-half mm2+cast emitted first so the
    PSUM recycle chain ends on the smaller ACT-half cast; output DMA on the
    (otherwise idle) gpsimd queue.  Output block order within a group is
    lane-major (l*4+gq) -- host unpermutes."""
    import concourse.mybir as mybir
    import concourse.tile as tile
    from concourse.bacc import Bacc

    op = _register_expbits()
    f16 = mybir.dt.float16
    bf16 = mybir.dt.bfloat16
    f32 = mybir.dt.float32
    u16 = mybir.dt.uint16
    nblk = n_quads * 4
    ngroups = nblk // 16

    nc = Bacc()
    im = nc.declare_dram_parameter("in", [128, ngroups * GRP_COLS], f16,
                                   isOutput=False)
    om = nc.declare_dram_parameter("out", [128, nblk * 24], f16, isOutput=True)

    with tile.TileContext(nc) as tc, ExitStack() as ctx:
        ipool = ctx.enter_context(tc.tile_pool(name="ipool", bufs=ipool_bufs))
        apool = ctx.enter_context(tc.tile_pool(name="apool", bufs=apool_bufs))
        abpool = ctx.enter_context(tc.tile_pool(name="abpool", bufs=apool_bufs))
        opool = ctx.enter_context(tc.tile_pool(name="opool", bufs=opool_bufs))
        dpool = ctx.enter_context(tc.tile_pool(name="dummy", bufs=1))
        # Tile serialises ALL accessors of one PSUM tile, so the ACT half
        # (lanes 0-2) and DVE half (lane 3) live in separate pools; mm2
        # outputs + cast recycle through ps_d so ps_a frees right after its
        # ACTIVATE read (gram(t+2) never waits on mm2/cast of group t).
        pa_pool = ctx.enter_context(tc.tile_pool(name="psA", bufs=2, space="PSUM"))
        pd_pool = ctx.enter_context(tc.tile_pool(name="psD", bufs=2, space="PSUM"))

        # constants + ACT exp-table warm while the first input DMAs fly
        bias_t = dpool.tile([128, 1], f32)
        nc.vector.memset(bias_t, -LN2 * 127.5)
        kap = dpool.tile([128, DVE_COLS], f32)
        nc.vector.memset(kap, SRC1_K)
        dmy = dpool.tile([128, 8], f32)
        nc.vector.memset(dmy, 0.0)
        dmy2 = dpool.tile([128, 8], bf16)
        nc.scalar.activation(out=dmy2, in_=dmy,
                             func=mybir.ActivationFunctionType.Exp,
                             scale=LN2 / 128.0, bias=bias_t[:, 0:1])

        PREFETCH = ipool_bufs - 2
        tiles = {}

        def fetch(g):
            if g >= ngroups or g in tiles:
                return
            in_t = ipool.tile([128, GRP_COLS], f16)
            nc.sync.dma_start(out=in_t, in_=im[:, g * GRP_COLS:(g + 1) * GRP_COLS])
            tiles[g] = in_t

        OBATCH = 4                     # groups per output DMA (SWDGE drain cost)
        ostate = [None]                # current o_t staging tile

        def consume(st):
            """Device out position p = col//128 (lane-major); natural block
            id there is gq*4+l.  All mm2 outputs land in ps_d[0:384]."""
            ps_a, ps_d, a_t, ab_t, in_t, g = st
            v_t = in_t[:, 2 * GRP_Q:GRP_COLS].bitcast(bf16)

            def mm2(p):
                col = p * 128
                l, gq = col // 512, (col % 512) // 128
                if col >= ACT_COLS:
                    lhs = ab_t[:, col - ACT_COLS:col - ACT_COLS + 128].bitcast(bf16)
                else:
                    lhs = a_t[:, col:col + 128]
                nc.tensor.matmul(
                    out=ps_d[:, p * 24:(p + 1) * 24], lhsT=lhs,
                    rhs=v_t[:, (gq * 4 + l) * 24:(gq * 4 + l + 1) * 24],
                    start=True, stop=True,
                )

            for p in range(12, 16):        # DVE-half first (only needs ab_t)
                mm2(p)
            for p in range(12):
                mm2(p)
            pos = g % OBATCH
            if pos == 0:
                o_new = opool.tile([128, OBATCH * 384], f16)
                ostate[0] = o_new
            o_t = ostate[0]
            nc.vector.tensor_copy(out=o_t[:, pos * 384:(pos + 1) * 384],
                                  in_=ps_d[:, 0:384])
            if pos == OBATCH - 1 or g == ngroups - 1:
                g0 = g - pos
                nc.gpsimd.dma_start(out=om[:, g0 * 384:(g + 1) * 384],
                                    in_=o_t[:, 0:(pos + 1) * 384])

        fetch(0)
        fetch(1)
        fetched = [2]

        def topup(g):
            n = 0
            while (fetched[0] < ngroups and fetched[0] <= g + PREFETCH
                   and n < 2):
                fetch(fetched[0])
                fetched[0] += 1
                n += 1

        pend = None
        for g in range(ngroups):
            in_t = tiles.pop(g)
            q_t = in_t[:, 0:GRP_Q]
            k_t = in_t[:, GRP_Q:2 * GRP_Q]
            ps_a = pa_pool.tile([128, ACT_COLS], f32)
            ps_d = pd_pool.tile([128, DVE_COLS], f32)
            for gq in range(4):
                for l in range(4):
                    col = l * 512 + gq * 128
                    out_ap = (ps_a[:, col:col + 128] if col < ACT_COLS
                              else ps_d[:, col - ACT_COLS:col - ACT_COLS + 128])
                    nc.tensor.matmul(
                        out=out_ap,
                        lhsT=k_t[32 * l:32 * l + 32, gq * 128:(gq + 1) * 128],
                        rhs=q_t[32 * l:32 * l + 32, gq * 128:(gq + 1) * 128],
                        start=True, stop=True,
                        tile_position=(32 * l, 0),
                    )
            topup(g)
            a_t = apool.tile([128, ACT_COLS], bf16)
            nc.scalar.activation(
                out=a_t, in_=ps_a,
                func=mybir.ActivationFunctionType.Exp,
                scale=LN2 / 128.0, bias=bias_t[:, 0:1],
            )
            # consume BEFORE the dve exp: its cast then precedes dveexp(t)
            # in the DVE queue, so ps_d recycles a full period earlier
            if pend is not None:
                consume(pend)
            ab_t = abpool.tile([128, DVE_COLS], u16)
            nc.vector._custom_dve(
                op, out=ab_t, in0=ps_d, in1=kap,
                s0=C0_MAGIC, s1=C1_VERT, imm2=BETA_1P,
            )
            pend = (ps_a, ps_d, a_t, ab_t, in_t, g)
        consume(pend)
    nc.finalize()
    return nc


def build_nc(n_quads=NQUAD, ipool_bufs=3, apool_bufs=3, opool_bufs=3):
    """Build the per-core Bass module (same NEFF for all 8 cores).

    Constraints baked in:
    - One combined input DMA per super-tile (q|k|v).
    - Row-tiled (tile_position) matmuls sharing a PSUM *bank* crash the
      device -> the 4 lanes of a quad write 4 distinct banks: lanes 0-2
      into ps_act (3 banks), lane 3 into ps_dve (1 bank).
    - Every accessor of one PSUM tile is serialised by the framework,
      so ACT work and DVE work live on separate PSUM tiles.
    """
    import concourse.mybir as mybir
    import concourse.tile as tile
    from concourse.bacc import Bacc

    op1, op2 = _register_exp_ops()

    f16 = mybir.dt.float16
    f32 = mybir.dt.float32
    i32 = mybir.dt.int32
    nblk = n_quads * 4
    assert n_quads % SUPER_Q == 0
    n_super = n_quads // SUPER_Q

    nc = Bacc()
    im = nc.declare_dram_parameter("in", [128, n_super * SUP_COLS], f16,
                                   isOutput=False)
    om = nc.declare_dram_parameter("out", [128, nblk * 24], f16, isOutput=True)

    with tile.TileContext(nc) as tc, ExitStack() as ctx:
        ipool = ctx.enter_context(tc.tile_pool(name="ipool", bufs=ipool_bufs))
        apool = ctx.enter_context(tc.tile_pool(name="apool", bufs=apool_bufs))
        adpool = ctx.enter_context(tc.tile_pool(name="adpool", bufs=apool_bufs))
        ypool = ctx.enter_context(tc.tile_pool(name="ypool", bufs=apool_bufs))
        opool = ctx.enter_context(tc.tile_pool(name="opool", bufs=opool_bufs))
        pa_pool = ctx.enter_context(tc.tile_pool(name="psA", bufs=2, space="PSUM"))
        pd_pool = ctx.enter_context(tc.tile_pool(name="psD", bufs=2, space="PSUM"))

        def emit_mm2_and_drain(st):
            """Consume stage st = (ps_a, ps_d, aa, ad, v_t, og_base, b0):
            mm2 for all 16 blocks, then cast + DMA the outputs."""
            ps_a, ps_d, aa, ad, v_t, og4, b0 = st
            for bb in range(16):
                if bb < NACT_BLK:
                    gq, l = bb // 3, bb % 3
                    lhs = aa[:, (l * 512 + gq * 128):(l * 512 + gq * 128 + 128)]
                    out_ap = ps_a[:, bb * 24:bb * 24 + 24]
                else:
                    gq, l = bb - NACT_BLK, 3
                    lhs = ad[:, gq * 128:gq * 128 + 128]
                    out_ap = ps_d[:, (bb - NACT_BLK) * 24:
                                  (bb - NACT_BLK) * 24 + 24]
                b_local = (og4 + gq) * 4 + l
                nc.tensor.matmul(
                    out=out_ap,
                    lhsT=lhs,
                    rhs=v_t[:, b_local * 24:(b_local + 1) * 24],
                    start=True, stop=True,
                )
            o_t = opool.tile([128, 16 * 24], f16)
            if CAST_ACT > 0:
                nc.scalar.copy(
                    out=o_t[:, 0:CAST_ACT], in_=ps_a[:, 0:CAST_ACT])
                nc.vector.tensor_copy(
                    out=o_t[:, CAST_ACT:288], in_=ps_a[:, CAST_ACT:288])
            else:
                nc.vector.tensor_copy(out=o_t[:, 0:288], in_=ps_a[:, 0:288])
            nc.vector.tensor_copy(out=o_t[:, 288:384], in_=ps_d[:, 0:96])
            nc.sync.dma_start(out=om[:, b0 * 24:(b0 + 16) * 24], in_=o_t)

        pend = None                             # previous group's stage
        for s in range(n_super):
            in_t = ipool.tile([128, SUP_COLS], f16)
            c0 = s * SUP_COLS
            nc.sync.dma_start(out=in_t, in_=im[:, c0:c0 + SUP_COLS])
            q_t = in_t[:, 0:SUP_Q_COLS]
            k_t = in_t[:, SUP_Q_COLS:2 * SUP_Q_COLS]
            v_t = in_t[:, 2 * SUP_Q_COLS:SUP_COLS]

            for og in range(4):                 # 4 groups of 4 quads (16 blk)
                ps_a = pa_pool.tile([128, 1536], f32)   # banks for lanes 0-2
                ps_d = pd_pool.tile([128, 512], f32)    # bank for lane 3
                for gq in range(4):             # ACT lanes first
                    qd = og * 4 + gq
                    for l in range(3):
                        col = l * 512 + gq * 128
                        nc.tensor.matmul(
                            out=ps_a[:, col:col + 128],
                            lhsT=k_t[32 * l:32 * l + 32, qd * 128:(qd + 1) * 128],
                            rhs=q_t[32 * l:32 * l + 32, qd * 128:(qd + 1) * 128],
                            start=True, stop=True,
                            tile_position=(32 * l, 0),
                        )
                for gq in range(4):             # DVE lane last
                    qd = og * 4 + gq
                    nc.tensor.matmul(
                        out=ps_d[:, gq * 128:gq * 128 + 128],
                        lhsT=k_t[96:128, qd * 128:(qd + 1) * 128],
                        rhs=q_t[96:128, qd * 128:(qd + 1) * 128],
                        start=True, stop=True,
                        tile_position=(96, 0),
                    )
                aa = apool.tile([128, 1536], f16)
                ad = adpool.tile([128, 512], f16)
                y_t = ypool.tile([128, 512], f32)
                nc.scalar.activation(
                    out=aa, in_=ps_a,
                    func=mybir.ActivationFunctionType.Exp,
                    scale=LN2,
                )
                nc.vector._custom_dve(
                    op1, out=y_t.bitcast(i32), in0=ps_d,
                    s0=DVE_M, s1=DVE_CLO, imm2=DVE_E23,
                )
                nc.vector._custom_dve(
                    op2, out=ad, in0=ps_d,
                    in1=y_t, s0=DVE_M, s1=DVE_C, imm2=DVE_A,
                )
                if pend is not None:            # consume the PREVIOUS group
                    emit_mm2_and_drain(pend)
                pend = (ps_a, ps_d, aa, ad, v_t, og * 4, (s * 4 + og) * 16)
        emit_mm2_and_drain(pend)
    nc.finalize()
    return nc


def build_nc_allact(n_quads=NQUAD, ipool_bufs=8, apool_bufs=3, opool_bufs=3):
    """Single-PSUM-pool variant: ALL exp on ScalarE (one 2048-col ACT
    instr per group keeps the scalar queue saturated), casts on VectorE,
    mm2 + drain software-pipelined one group behind the fills so every
    wait is pre-posted when its consumer reaches the queue head.
    Input arrives in per-group chunks (1408 cols) DMA'd several groups
    ahead so super-boundary stalls disappear."""
    import concourse.mybir as mybir
    import concourse.tile as tile
    from concourse.bacc import Bacc

    f16 = mybir.dt.float16
    f32 = mybir.dt.float32
    nblk = n_quads * 4
    ngroups = nblk // 16

    nc = Bacc()
    im = nc.declare_dram_parameter("in", [128, ngroups * GRP_COLS], f16,
                                   isOutput=False)
    om = nc.declare_dram_parameter("out", [128, nblk * 24], f16, isOutput=True)

    with tile.TileContext(nc) as tc, ExitStack() as ctx:
        ipool = ctx.enter_context(tc.tile_pool(name="ipool", bufs=ipool_bufs))
        apool = ctx.enter_context(tc.tile_pool(name="apool", bufs=apool_bufs))
        opool = ctx.enter_context(tc.tile_pool(name="opool", bufs=opool_bufs))
        dpool = ctx.enter_context(tc.tile_pool(name="dummy", bufs=1))
        ps1pool = ctx.enter_context(tc.tile_pool(name="ps1", bufs=2, space="PSUM"))

        # warm the exp ACT table while the first input DMAs are in flight
        dmy = dpool.tile([128, 8], f32)
        nc.vector.memset(dmy, 0.0)
        dmy2 = dpool.tile([128, 8], f16)
        nc.scalar.activation(out=dmy2, in_=dmy,
                             func=mybir.ActivationFunctionType.Exp)

        PREFETCH = ipool_bufs - 2
        tiles = {}

        def fetch(g):
            if g >= ngroups or g in tiles:
                return
            in_t = ipool.tile([128, GRP_COLS], f16)
            nc.sync.dma_start(out=in_t, in_=im[:, g * GRP_COLS:(g + 1) * GRP_COLS])
            tiles[g] = in_t

        def consume(st):
            ps1, a_t, in_t, b0 = st
            v_t = in_t[:, 2 * GRP_Q:GRP_COLS]
            for bb in range(16):
                gq, l = bb // 4, bb % 4
                acol = l * 512 + gq * 128
                nc.tensor.matmul(
                    out=ps1[:, bb * 24:(bb + 1) * 24],
                    lhsT=a_t[:, acol:acol + 128],
                    rhs=v_t[:, (gq * 4 + l) * 24:(gq * 4 + l + 1) * 24],
                    start=True, stop=True,
                )
            o_t = opool.tile([128, 16 * 24], f16)
            nc.vector.tensor_copy(out=o_t, in_=ps1[:, 0:384])
            nc.sync.dma_start(out=om[:, b0 * 24:(b0 + 16) * 24], in_=o_t)

        # Stagger the prefetch buildup: only groups 0-1 before the loop so
        # group 0's chunk isn't queued behind 2MB of lookahead traffic;
        # depth then grows by up to 2 fetches per group until PREFETCH.
        fetch(0)
        fetch(1)
        fetched = [2]

        def topup(g):
            n = 0
            while (fetched[0] < ngroups and fetched[0] <= g + PREFETCH
                   and n < 2):
                fetch(fetched[0])
                fetched[0] += 1
                n += 1

        pend = None
        for g in range(ngroups):
            in_t = tiles.pop(g)
            q_t = in_t[:, 0:GRP_Q]
            k_t = in_t[:, GRP_Q:2 * GRP_Q]
            ps1 = ps1pool.tile([128, 2048], f32)
            for gq in range(4):
                for l in range(4):
                    col = l * 512 + gq * 128
                    nc.tensor.matmul(
                        out=ps1[:, col:col + 128],
                        lhsT=k_t[32 * l:32 * l + 32, gq * 128:(gq + 1) * 128],
                        rhs=q_t[32 * l:32 * l + 32, gq * 128:(gq + 1) * 128],
                        start=True, stop=True,
                        tile_position=(32 * l, 0),
                    )
            topup(g)
            a_t = apool.tile([128, 2048], f16)
            nc.scalar.activation(
                out=a_t, in_=ps1,
                func=mybir.ActivationFunctionType.Exp,
                scale=LN2,
            )
            if pend is not None:
                consume(pend)
            pend = (ps1, a_t, in_t, g * 16)
        consume(pend)
    nc.finalize()
    return nc


def build_nc_split2(n_quads=NQUAD, ipool_bufs=8, apool_bufs=3, opool_bufs=3):
    """Per-group-DMA + software-pipelined variant with the exp split
    across ScalarE (lanes 0-2, 3 PSUM banks) and VectorE custom-DVE
    (lane 3, 1 PSUM bank).  Output casts: ps_a's 288 cols on ACT (its
    own pool), ps_d's 96 cols on DVE."""
    import concourse.mybir as mybir
    import concourse.tile as tile
    from concourse.bacc import Bacc

    op1, op2 = _register_exp_ops()
    f16 = mybir.dt.float16
    f32 = mybir.dt.float32
    i32 = mybir.dt.int32
    nblk = n_quads * 4
    ngroups = nblk // 16

    nc = Bacc()
    im = nc.declare_dram_parameter("in", [128, ngroups * GRP_COLS], f16,
                                   isOutput=False)
    om = nc.declare_dram_parameter("out", [128, nblk * 24], f16, isOutput=True)

    with tile.TileContext(nc) as tc, ExitStack() as ctx:
        ipool = ctx.enter_context(tc.tile_pool(name="ipool", bufs=ipool_bufs))
        apool = ctx.enter_context(tc.tile_pool(name="apool", bufs=apool_bufs))
        adpool = ctx.enter_context(tc.tile_pool(name="adpool", bufs=apool_bufs))
        ypool = ctx.enter_context(tc.tile_pool(name="ypool", bufs=apool_bufs))
        opool = ctx.enter_context(tc.tile_pool(name="opool", bufs=opool_bufs))
        dpool = ctx.enter_context(tc.tile_pool(name="dummy", bufs=1))
        pa_pool = ctx.enter_context(tc.tile_pool(name="psA", bufs=2, space="PSUM"))
        pd_pool = ctx.enter_context(tc.tile_pool(name="psD", bufs=2, space="PSUM"))

        # warm the exp ACT table while the first input DMA is in flight
        dmy = dpool.tile([128, 8], f32)
        nc.vector.memset(dmy, 0.0)
        dmy2 = dpool.tile([128, 8], f16)
        nc.scalar.activation(out=dmy2, in_=dmy,
                             func=mybir.ActivationFunctionType.Exp)

        PREFETCH = ipool_bufs - 2
        tiles = {}

        def fetch(g):
            if g >= ngroups or g in tiles:
                return
            in_t = ipool.tile([128, GRP_COLS], f16)
            nc.sync.dma_start(out=in_t, in_=im[:, g * GRP_COLS:(g + 1) * GRP_COLS])
            tiles[g] = in_t

        def consume(st):
            ps_a, ps_d, aa, ad, in_t, b0 = st
            v_t = in_t[:, 2 * GRP_Q:GRP_COLS]
            for bb in range(16):
                if bb < NACT_BLK:
                    gq, l = bb // 3, bb % 3
                    lhs = aa[:, (l * 512 + gq * 128):(l * 512 + gq * 128 + 128)]
                    out_ap = ps_a[:, bb * 24:bb * 24 + 24]
                else:
                    gq, l = bb - NACT_BLK, 3
                    lhs = ad[:, gq * 128:gq * 128 + 128]
                    out_ap = ps_d[:, (bb - NACT_BLK) * 24:
                                  (bb - NACT_BLK) * 24 + 24]
                nc.tensor.matmul(
                    out=out_ap,
                    lhsT=lhs,
                    rhs=v_t[:, (gq * 4 + l) * 24:(gq * 4 + l + 1) * 24],
                    start=True, stop=True,
                )
            o_t = opool.tile([128, 16 * 24], f16)
            nc.scalar.copy(out=o_t[:, 0:96], in_=ps_a[:, 0:96])
            nc.vector.tensor_copy(out=o_t[:, 96:288], in_=ps_a[:, 96:288])
            nc.vector.tensor_copy(out=o_t[:, 288:384], in_=ps_d[:, 0:96])
            nc.sync.dma_start(out=om[:, b0 * 24:(b0 + 16) * 24], in_=o_t)

        for g in range(PREFETCH):
            fetch(g)
        pend = None
        for g in range(ngroups):
            in_t = tiles.pop(g)
            q_t = in_t[:, 0:GRP_Q]
            k_t = in_t[:, GRP_Q:2 * GRP_Q]
            ps_a = pa_pool.tile([128, 1536], f32)
            ps_d = pd_pool.tile([128, 512], f32)
            for gq in range(4):
                for l in range(3):
                    col = l * 512 + gq * 128
                    nc.tensor.matmul(
                        out=ps_a[:, col:col + 128],
                        lhsT=k_t[32 * l:32 * l + 32, gq * 128:(gq + 1) * 128],
                        rhs=q_t[32 * l:32 * l + 32, gq * 128:(gq + 1) * 128],
                        start=True, stop=True,
                        tile_position=(32 * l, 0),
                    )
            for gq in range(4):
                nc.tensor.matmul(
                    out=ps_d[:, gq * 128:gq * 128 + 128],
                    lhsT=k_t[96:128, gq * 128:(gq + 1) * 128],
                    rhs=q_t[96:128, gq * 128:(gq + 1) * 128],
                    start=True, stop=True,
                    tile_position=(96, 0),
                )
            fetch(g + PREFETCH)
            aa = apool.tile([128, 1536], f16)
            ad = adpool.tile([128, 512], f16)
            y_t = ypool.tile([128, 512], f32)
            nc.scalar.activation(
                out=aa, in_=ps_a,
                func=mybir.ActivationFunctionType.Exp,
                scale=LN2,
            )
            nc.vector._custom_dve(
                op1, out=y_t.bitcast(i32), in0=ps_d,
                s0=DVE_M, s1=DVE_CLO, imm2=DVE_E23,
            )
            nc.vector._custom_dve(
                op2, out=ad, in0=ps_d,
                in1=y_t, s0=DVE_M, s1=DVE_C, imm2=DVE_A,
            )
            if pend is not None:
                consume(pend)
            pend = (ps_a, ps_d, aa, ad, in_t, g * 16)
        consume(pend)
    nc.finalize()
    return nc


def _get_nc(n_quads=NQUAD):
    key = (n_quads, MODE)
    if key not in _NC_CACHE:
        if MODE == "split1p":
            _NC_CACHE[key] = build_nc_split1p(n_quads)
        elif MODE == "allact":
            _NC_CACHE[key] = build_nc_allact(n_quads)
        elif MODE == "split2":
            _NC_CACHE[key] = build_nc_split2(n_quads)
        else:
            _NC_CACHE[key] = build_nc(n_quads)
    return _NC_CACHE[key]


# ---------------- host-side preparation ----------------

def _sort_indices(query, key, combined_shifts, alpha):
    """Replicate the reference's hash + argsort with jax on CPU.

    Uses the exact same jnp ops the reference uses so the fp32 values
    (and therefore the argsort permutations) match bit-for-bit.
    """
    import jax
    import jax.numpy as jnp

    cpu = jax.devices("cpu")[0]
    with jax.default_device(cpu):
        q = jnp.asarray(query)
        k = jnp.asarray(key)
        al = jnp.asarray(alpha)
        cs_i = jnp.asarray(combined_shifts)
        q_hashed = jnp.einsum('hnd,hdr->rhn', q, al)
        k_hashed = jnp.einsum('hnd,hdr->rhn', k, al)
        max_shift = jnp.maximum(q_hashed.max(-1, keepdims=True),
                                k_hashed.max(-1, keepdims=True))
        min_shift = jnp.minimum(q_hashed.min(-1, keepdims=True),
                                k_hashed.min(-1, keepdims=True))
        hash_shift = max_shift - min_shift
        cs = cs_i.astype(q_hashed.dtype) * hash_shift
        q_pos = np.asarray(jnp.argsort(q_hashed + cs, axis=-1))
        k_pos = np.asarray(jnp.argsort(k_hashed + cs, axis=-1))
    return q_pos, k_pos


def _split16(x):
    hi = x.astype(np.float16)
    lo = (x - hi.astype(np.float32)).astype(np.float16)
    return hi, lo


def _build_stack(s_qk, is_k, scale=SQRT_LOG2E, offset=0.0):
    """(UNITS*NB, 128, 27) f32 -> (UNITS*NB, 32, 128) fp16 stack.

    Data rows are pre-scaled by `scale` so the on-device Gram is
    scale^2 * (-0.5*||q-k||^2); the norm rows use the scaled fp16 data
    and carry `offset` (split1p bakes +8160 per side into the Gram).
    """
    nblk = s_qk.shape[0]
    hi = (s_qk * np.float32(scale)).astype(np.float16)  # (b, i, d)
    sqm = -0.5 * np.einsum('bid,bid->bi', hi.astype(np.float32),
                           hi.astype(np.float32)) + np.float32(offset)
    sq_hi, sq_lo = _split16(sqm)
    st = np.zeros((nblk, KROWS, BLOCK), np.float16)
    st[:, :D_QK, :] = hi.transpose(0, 2, 1)            # rows 0-26: x^T
    if is_k:
        st[:, 27, :] = 1.0                             # pair of q's sq rows
        st[:, 28, :] = 1.0
        st[:, 29, :] = sq_hi                           # -0.5*||k||^2 hi
        st[:, 30, :] = sq_lo
    else:
        st[:, 27, :] = sq_hi                           # -0.5*||q||^2 hi
        st[:, 28, :] = sq_lo
        st[:, 29, :] = 1.0                             # pair of k's sq rows
        st[:, 30, :] = 1.0
    return st


def _pack_core(stack_blocks):
    """(768, 32, 128) -> (128, 192*128): partition = lane*32+row,
    free = quad*128 + col."""
    return (stack_blocks.reshape(NQUAD, 4, KROWS, BLOCK)
            .transpose(1, 2, 0, 3)
            .reshape(128, NQUAD * BLOCK))


# om block order within each 16-block group: 12 ACT blocks (lanes 0-2 in
# (quad, lane) order) then 4 DVE blocks (lane 3).  _OM_PERM[bb] = b_local
# offset within the group's 16 blocks.
_OM_PERM = np.array(
    [q * 4 + l for q in range(4) for l in range(3)] +
    [q * 4 + 3 for q in range(4)], dtype=np.int64)


def prepare_in_maps(query, key, value, combined_shifts, alpha):
    query = np.ascontiguousarray(np.asarray(query), dtype=np.float32)
    key = np.ascontiguousarray(np.asarray(key), dtype=np.float32)
    value = np.ascontiguousarray(np.asarray(value), dtype=np.float32)
    combined_shifts = np.asarray(combined_shifts)
    alpha = np.asarray(alpha, dtype=np.float32)

    q_pos, k_pos = _sort_indices(query, key, combined_shifts, alpha)

    h_idx = np.arange(N_HEADS)[None, :, None]
    s_query = query[h_idx, q_pos].reshape(UNITS * NB, BLOCK, D_QK)
    s_key = key[h_idx, k_pos].reshape(UNITS * NB, BLOCK, D_QK)
    s_value = value[h_idx, k_pos].reshape(UNITS * NB, BLOCK, DIM_PER_HEAD)

    if MODE == "split1p":
        qstack = _build_stack(s_query, is_k=False, scale=SC2, offset=G_OFS / 2)
        kstack = _build_stack(s_key, is_k=True, scale=SC2, offset=G_OFS / 2)
        # v as bf16 bit patterns carried in the f16-typed input buffer
        vb = np.ascontiguousarray(s_value, np.float32).view(np.uint32)
        vb = ((vb + 0x7FFF + ((vb >> 16) & 1)) >> 16).astype(np.uint16)
        v16 = vb.view(np.float16)
    else:
        qstack = _build_stack(s_query, is_k=False)
        kstack = _build_stack(s_key, is_k=True)
        v16 = s_value.astype(np.float16)

    in_maps = []
    for c in range(N_CORES):
        b0, b1 = c * NBLK, (c + 1) * NBLK
        qp = _pack_core(qstack[b0:b1])              # [128, NQUAD*128]
        kp = _pack_core(kstack[b0:b1])
        vp = v16[b0:b1].transpose(1, 0, 2).reshape(128, NBLK * 24)
        combined = np.empty((128, NGROUPS * GRP_COLS), np.float16)
        for g in range(NGROUPS):
            c0 = g * GRP_COLS
            combined[:, c0:c0 + GRP_Q] = qp[:, g * GRP_Q:(g + 1) * GRP_Q]
            combined[:, c0 + GRP_Q:c0 + 2 * GRP_Q] = \
                kp[:, g * GRP_Q:(g + 1) * GRP_Q]
            combined[:, c0 + 2 * GRP_Q:c0 + GRP_COLS] = \
                vp[:, g * GRP_V:(g + 1) * GRP_V]
        in_maps.append({"in": combined})
    return in_maps


def assemble_output(results):
    """results: list of 8 dicts with 'out' [128, 768*24] f16 in the
    permuted (ACT-blocks-first) group order."""
    ngroups = NBLK // 16
    if MODE == "allact":
        om_perm = np.arange(16)
    elif MODE == "split1p":
        # device col p = l*4+gq holds natural block gq*4+l
        om_perm = np.array([(p % 4) * 4 + p // 4 for p in range(16)],
                          dtype=np.int64)
    else:
        om_perm = _OM_PERM
    perm = (np.arange(ngroups)[:, None] * 16 + om_perm[None, :]).ravel()
    inv = np.empty_like(perm)
    inv[perm] = np.arange(NBLK)
    out = np.empty((UNITS, NB, BLOCK, DIM_PER_HEAD), np.float32)
    for c in range(N_CORES):
        so = np.asarray(results[c]["out"]).astype(np.float32)
        so = so.reshape(128, NBLK, 24)[:, inv, :]
        out[c * UPC:(c + 1) * UPC] = (
            so.transpose(1, 0, 2).reshape(UPC, NB, BLOCK, DIM_PER_HEAD))
    return out.reshape(N_HASHES, N_HEADS, NB, BLOCK, DIM_PER_HEAD)


def run(query, key, value, combined_shifts, alpha, trace=False):
    from concourse.bass_utils import run_bass_kernel_spmd

    in_maps = prepare_in_maps(query, key, value, combined_shifts, alpha)
    nc = _get_nc()
    res = run_bass_kernel_spmd(
        nc, in_maps, core_ids=list(range(N_CORES)), trace=trace)
    out = assemble_output(res.results)
    return out, res


def kernel(query, key, value, combined_shifts, alpha):
    out, _ = run(query, key, value, combined_shifts, alpha,
                 trace=bool(int(os.environ.get("HEPT_TRACE", "0"))))
    return out

